# revision 16
# baseline (speedup 1.0000x reference)
"""Multi-head attention (B=4, S=2048, D=1024, H=16) on 8 Trainium2 cores.

Sharding: core c -> (batch b=c//2, query-half hq=c%2). Each core computes
K/V projections for its batch's full sequence (no collectives needed) and
attention + output projection for its 1024 query rows.

Device dataflow (activations kept transposed, [feature, seq], except ctx):
  kT[e,s]    = WkT.T-contract  (lhsT=WkT[d,e] tiles, rhs=xT[d,s])
  qTz[h]     = per-head zero-padded q [128, SQ]: head h's 64 dims at
               partitions (h%2)*64, rest zero.  Scores then contract over
               the full K=128 partitions (K=64 matmuls stream at half rate
               on trn2; zero rows make K=128 exact and full speed).
  v[s,e]     = lhsT=xT[d,s] tiles, rhs=WvT[d,e]  (+bias via DVE add of a
               partition-broadcast bv)
  per (q-chunk qc of 512, head h, k-pair kh):
    scoresT[k,q] = kT.T-contract qTz  (2 matmuls/kh -> [128,1024] psum)
    expT = ScalarE Exp(scale=0.125) -> bf16 sbuf
    flipped ctx: for each 128-q subtile qi: ctq[q,hd+1] += expT-slice
               (stationary, M=128) @ vv[kt][:,h,:] (moving, N=65);
               col 64 = softmax denominator (ones column of vv)
    norm: reciprocal_approx_fast [128,1] + tensor_scalar_mul (per-q denom
               is per-PARTITION in this layout), PE-transpose back to
               ctxn[d, q]
  outT[e,q]  = WoT.T-contract ctxn  (bias bo added host-side)
Projections for e-tiles >= 1 and v-chunks >= 1 are emitted as PE "filler"
groups inside the attention loop so the PE works while ScalarE exps pace
the attention pipeline.
Host: out[b, hq*1024:(hq+1)*1024, :] = outT.T + bo
"""

import numpy as np
import ml_dtypes

import concourse.bacc as bacc
import concourse.tile as tile
from concourse import mybir
from concourse.bass_utils import run_bass_kernel_spmd
from concourse.masks import make_identity

B, S, D = 4, 2048, 1024
H, HD = 16, 64
SQ = 1024          # query rows per core
NDT = D // 128     # 8 d-tiles
NET = D // 128     # 8 e-tiles
NKT = S // 128     # 16 k-tiles
NST = S // 128     # 16 s-tiles
NQC = SQ // 512    # 2 q-chunks per core
BF16 = mybir.dt.bfloat16
F32 = mybir.dt.float32
SCALE = 1.0 / 8.0  # 1/sqrt(HD)

_NC_CACHE = None


def build_nc():
    nc = bacc.Bacc(None, target_bir_lowering=False, debug=True)

    xT_d = nc.declare_dram_parameter("xT", [D, S], BF16, isOutput=False)
    WqT_d = nc.declare_dram_parameter("WqT", [D, D], BF16, isOutput=False)
    WkT_d = nc.declare_dram_parameter("WkT", [D, D], BF16, isOutput=False)
    WvT_d = nc.declare_dram_parameter("WvT", [D, D], BF16, isOutput=False)
    WoT_d = nc.declare_dram_parameter("WoT", [D, D], BF16, isOutput=False)
    bqt_d = nc.declare_dram_parameter("bqt", [128, NET], F32, isOutput=False)
    bkt_d = nc.declare_dram_parameter("bkt", [128, NET], F32, isOutput=False)
    bvr_d = nc.declare_dram_parameter("bvr", [1, D], F32, isOutput=False)
    outT_d = nc.declare_dram_parameter("outT", [D, SQ], F32, isOutput=True)

    VC = 256           # v-projection chunk width (4 heads per chunk)
    NVC = D // VC      # 4 chunks

    with tile.TileContext(nc) as tc:
        with tc.tile_pool(name="resident", bufs=1) as res:
            # ---- resident SBUF tensors ----
            kT = [res.tile([128, S], BF16, name=f"kT{t}", tag=f"kT{t}")
                  for t in range(NET)]
            qTz = [res.tile([128, SQ], BF16, name=f"qTz{h}", tag=f"qTz{h}")
                   for h in range(H)]
            vv = [res.tile([128, H, HD + 1], BF16, name=f"v{t}", tag=f"v{t}")
                  for t in range(NST)]
            ctxn = [[res.tile([128, 512], BF16, name=f"ctxn{qc}_{t}",
                              tag=f"ctxn{qc}_{t}") for t in range(NDT)]
                    for qc in range(NQC)]
            Wo_t = [res.tile([128, D], BF16, name=f"Wo{t}", tag=f"Wo{t}")
                    for t in range(NDT)]
            xT = [res.tile([128, S], BF16, name=f"xT{t}", tag=f"xT{t}")
                  for t in range(NDT)]
            bq_dma = res.tile([128, NET], F32, tag="bq_dma")
            bk_dma = res.tile([128, NET], F32, tag="bk_dma")
            bq_sb = res.tile([128, NET], F32, tag="bq_sb")
            bk_sb = res.tile([128, NET], F32, tag="bk_sb")
            bv_sb = res.tile([1, D], F32, tag="bv_sb")
            bv_bc = res.tile([128, D], F32, tag="bv_bc")
            ident = res.tile([128, 128], BF16, tag="ident")

            nc.sync.dma_start(out=bq_dma, in_=bqt_d[:, :])
            nc.sync.dma_start(out=bk_dma, in_=bkt_d[:, :])
            nc.sync.dma_start(out=bv_sb, in_=bvr_d[:, :])
            # TensorScalarPtr has a single sync-wait slot; route the biases
            # through DVE once so later readers rely on program order.
            nc.vector.tensor_copy(out=bq_sb, in_=bq_dma)
            nc.vector.tensor_copy(out=bk_sb, in_=bk_dma)
            nc.gpsimd.partition_broadcast(bv_bc, bv_sb[0:1, :])
            make_identity(nc, ident)
            for h in range(H):
                z0 = 64 if h % 2 == 0 else 0
                nc.vector.memset(qTz[h][z0:z0 + 64, :], 0.0)
            for t in range(NST):
                # only the denominator column; cols 0:HD are overwritten
                nc.vector.memset(vv[t][:, :, HD:HD + 1], 1.0)

            for t in range(NDT):
                nc.sync.dma_start(out=xT[t], in_=xT_d[t * 128:(t + 1) * 128, :])
            for t in range(NDT):
                nc.sync.dma_start(out=Wo_t[t], in_=WoT_d[t * 128:(t + 1) * 128, :])

            with tc.tile_pool(name="p2", bufs=1) as p2:
                psum_src = {}

                def proj_ps():
                    return psum_src["pool"].tile(
                        [128, 512], F32, name="ps", tag=psum_src["tag"],
                        bufs=psum_src["bufs"])

                # ---------- projection emitters (also used as fillers) ----
                # weight SLICES are DMA-streamed per e-tile/chunk so that
                # three full weight sets never have to live in SBUF at once
                wk_cache = {}
                wq_cache = {}
                wv_cache = {}

                def w_slices(cache, key, W_d, c0, c1, tag, bufs):
                    if key not in cache:
                        ws = []
                        for dt in range(NDT):
                            wt = p2.tile([128, c1 - c0], BF16,
                                         name=f"{tag}{dt}", tag=tag,
                                         bufs=bufs)
                            nc.sync.dma_start(
                                out=wt, in_=W_d[dt * 128:(dt + 1) * 128,
                                                c0:c1])
                            ws.append(wt)
                        cache.clear()
                        cache[key] = ws
                    return cache[key]

                def emit_k_group(et, sc):
                    ws = w_slices(wk_cache, ("k", et), WkT_d,
                                  et * 128, (et + 1) * 128, "wks", 18)
                    ps = proj_ps()
                    for dt in range(NDT):
                        nc.tensor.matmul(
                            ps, ws[dt],
                            xT[dt][:, sc * 512: sc * 512 + 512],
                            start=(dt == 0), stop=(dt == NDT - 1))
                    nc.vector.tensor_scalar_add(
                        out=kT[et][:, sc * 512:(sc + 1) * 512],
                        in0=ps,
                        scalar1=bk_sb[:, et:et + 1])

                def emit_q_group(et, sc):
                    ws = w_slices(wq_cache, ("q", et), WqT_d,
                                  et * 128, (et + 1) * 128, "wqs", 18)
                    ps = proj_ps()
                    for dt in range(NDT):
                        nc.tensor.matmul(
                            ps, ws[dt],
                            xT[dt][:, sc * 512: sc * 512 + 512],
                            start=(dt == 0), stop=(dt == NDT - 1))
                    sl = slice(sc * 512, (sc + 1) * 512)
                    nc.vector.tensor_scalar_add(
                        out=qTz[2 * et][0:64, sl],
                        in0=ps[0:64, :],
                        scalar1=bq_sb[0:64, et:et + 1])
                    nc.vector.tensor_scalar_add(
                        out=qTz[2 * et + 1][64:128, sl],
                        in0=ps[64:128, :],
                        scalar1=bq_sb[64:128, et:et + 1])

                def emit_v_group(st, c):
                    # v chunk c covers e-columns [c*VC, (c+1)*VC) = 4 heads
                    ws = w_slices(wv_cache, ("v", c), WvT_d,
                                  c * VC, (c + 1) * VC, "wvs", 18)
                    psw = proj_ps()
                    ps = psw[:, 0:VC]
                    for dt in range(NDT):
                        nc.tensor.matmul(
                            ps,
                            xT[dt][:, st * 128:(st + 1) * 128],
                            ws[dt],
                            start=(dt == 0), stop=(dt == NDT - 1))
                    nh = VC // HD
                    nc.vector.tensor_add(
                        out=vv[st][:, c * nh:(c + 1) * nh, 0:HD],
                        in0=ps.rearrange("p (h d) -> p h d", h=nh),
                        in1=bv_bc[:, c * VC:(c + 1) * VC].rearrange(
                            "p (h d) -> p h d", h=nh))

                # ---------- upfront: first e-tile + first v chunk ----------
                with tc.psum_pool(name="pf", bufs=1) as pf:
                    psum_src.update(pool=pf, tag="pfg", bufs=2)
                    for sc in range(S // 512):
                        emit_k_group(0, sc)
                    for sc in range(NQC):
                        emit_q_group(0, sc)
                    for st in range(NST):
                        emit_v_group(st, 0)
                _cms = [tc.psum_pool(name="sp", bufs=2),
                        tc.psum_pool(name="cp", bufs=2),
                        tc.psum_pool(name="op", bufs=1),
                        tc.psum_pool(name="tp", bufs=1)]
                sp, cp, op, tp = [cm.__enter__() for cm in _cms]
                psum_src.update(pool=op, tag="op", bufs=1)

                fillers = []
                for et in range(1, NET):
                    for sc in range(S // 512):
                        fillers.append((emit_k_group, et, sc))
                    for sc in range(NQC):
                        fillers.append((emit_q_group, et, sc))
                    if et in (2, 4, 6):
                        c = et // 2
                        for st in range(NST):
                            fillers.append((emit_v_group, st, c))
                fillers = fillers[::-1]  # pop from the end

                # ---------- attention ----------
                def emit_sc(qc, h, kh):
                    ht = h // 2
                    sc_ps = sp.tile([128, 1024], F32, name="sc_ps",
                                    tag="sc", bufs=2)
                    for j in range(2):
                        kt = kh * 2 + j
                        nc.tensor.matmul(
                            sc_ps[:, j * 512:(j + 1) * 512],
                            kT[ht][:, kt * 128:(kt + 1) * 128],
                            qTz[h][:, qc * 512:(qc + 1) * 512],
                            start=True, stop=True)
                    return sc_ps

                def emit_outproj(qc_o, et):
                    ps = op.tile([128, 512], F32, name="ops", tag="op",
                                 bufs=1)
                    for dt in range(NDT):
                        nc.tensor.matmul(
                            ps,
                            Wo_t[dt][:, et * 128:(et + 1) * 128],
                            ctxn[qc_o][dt][:, :],
                            start=(dt == 0), stop=(dt == NDT - 1))
                    osb = p2.tile([128, 512], F32, name="osb", tag="osb",
                                  bufs=2)
                    nc.vector.tensor_copy(out=osb, in_=ps)
                    nc.sync.dma_start(
                        out=outT_d[et * 128:(et + 1) * 128,
                                   qc_o * 512:(qc_o + 1) * 512],
                        in_=osb)

                def emit_norm(ctq_ps, qi, qc, h):
                    # per-q denominators sit per-PARTITION in flipped layout
                    ht, hp = h // 2, (h % 2) * 64
                    inv = p2.tile([128, 1], F32, name="inv", tag="inv",
                                  bufs=3)
                    nc.vector.reciprocal_approx_fast(
                        inv, ctq_ps[:, qi, HD:HD + 1])
                    ctqn = p2.tile([128, HD], BF16, name="ctqn", tag="ctqn",
                                   bufs=3)
                    nc.vector.tensor_scalar_mul(
                        out=ctqn, in0=ctq_ps[:, qi, 0:HD], scalar1=inv)
                    tp_ps = tp.tile([HD, 128], BF16, name="tp_ps", tag="tp",
                                    bufs=1)
                    nc.tensor.transpose(tp_ps, ctqn, ident[:, :])
                    nc.vector.tensor_copy(
                        out=ctxn[qc][ht][hp:hp + HD, qi * 128:(qi + 1) * 128],
                        in_=tp_ps)

                iters = [(qc, h, kh)
                         for qc in range(NQC)
                         for h in range(H)
                         for kh in range(NKT // 2)]
                op_queue = []
                norm_queue = []
                ctq_state = {"ps": None}

                def emit_ctx_step(expT, qc, h, kh):
                    # flipped ctx: expT slice stationary (M=128 q), v moving
                    # (N=65); 4 q-subtiles accumulate in one psum bank
                    if kh == 0:
                        ctq_state["ps"] = cp.tile([128, 4, HD + 1], F32,
                                                  name="ctq", tag="ctq",
                                                  bufs=2)
                    ctq_ps = ctq_state["ps"]
                    for qi in range(4):
                        for j in range(2):
                            kt = kh * 2 + j
                            # start only on the bank's first write: the
                            # start bit marks the WHOLE 2KB bank pending-
                            # zero, so sibling qi regions must not re-set it
                            nc.tensor.matmul(
                                ctq_ps[:, qi, :],
                                expT[:, j * 512 + qi * 128:
                                     j * 512 + qi * 128 + 128],
                                vv[kt][:, h, :],
                                start=(kt == 0 and qi == 0),
                                stop=(kt == NKT - 1))
                    if kh == NKT // 2 - 1:
                        for qi in range(4):
                            norm_queue.append((ctq_ps, qi, qc, h))
                        if qc == 0 and h == H - 1:
                            op_queue.extend((0, et) for et in range(NET))

                sc_next = emit_sc(*iters[0])
                delayed = []
                for i, (qc, h, kh) in enumerate(iters):
                    sc_ps = sc_next
                    expT = p2.tile([128, 1024], BF16, name="expT",
                                   tag="expT", bufs=4)
                    nc.scalar.activation(
                        expT, sc_ps,
                        mybir.ActivationFunctionType.Exp,
                        scale=SCALE)
                    if i + 1 < len(iters):
                        sc_next = emit_sc(*iters[i + 1])
                    delayed.append((expT, qc, h, kh))
                    if len(delayed) > 1:
                        emit_ctx_step(*delayed.pop(0))
                    if norm_queue:
                        emit_norm(*norm_queue.pop(0))
                    if fillers:
                        fn, *args = fillers.pop()
                        fn(*args)
                    if kh == 6 and op_queue and h % 2 == 1:
                        emit_outproj(*op_queue.pop(0))
                while delayed:
                    emit_ctx_step(*delayed.pop(0))
                while norm_queue:
                    emit_norm(*norm_queue.pop(0))
                for args in op_queue:
                    emit_outproj(*args)
                for et in range(NET):
                    emit_outproj(1, et)
                for cm in reversed(_cms):
                    cm.__exit__(None, None, None)
    nc.compile()
    return nc


def _get_nc():
    global _NC_CACHE
    if _NC_CACHE is None:
        _NC_CACHE = build_nc()
    return _NC_CACHE


def _prep_maps(x, Wq, bq, Wk, bk, Wv, bv, Wo):
    bf = ml_dtypes.bfloat16
    WqT = np.ascontiguousarray(Wq.T).astype(bf)
    WkT = np.ascontiguousarray(Wk.T).astype(bf)
    WvT = np.ascontiguousarray(Wv.T).astype(bf)
    WoT = np.ascontiguousarray(Wo.T).astype(bf)
    bqt = np.ascontiguousarray(bq.reshape(NET, 128).T).astype(np.float32)
    bkt = np.ascontiguousarray(bk.reshape(NET, 128).T).astype(np.float32)
    bvr = np.ascontiguousarray(bv.reshape(1, D)).astype(np.float32)
    in_maps = []
    for c in range(8):
        b, hq = c // 2, c % 2
        xTb = np.ascontiguousarray(x[b].T).astype(bf)  # [D, S]
        if hq == 1:
            # rotate so local query half sits at columns [0, SQ)
            xTb = np.ascontiguousarray(
                np.concatenate([xTb[:, SQ:], xTb[:, :SQ]], axis=1))
        in_maps.append(dict(xT=xTb, WqT=WqT, WkT=WkT, WvT=WvT, WoT=WoT,
                            bqt=bqt, bkt=bkt, bvr=bvr))
    return in_maps


def run(x, Wq, bq, Wk, bk, Wv, bv, Wo, bo, trace=False, **spmd_kwargs):
    nc = _get_nc()
    in_maps = _prep_maps(x, Wq, bq, Wk, bk, Wv, bv, Wo)
    res = run_bass_kernel_spmd(nc, in_maps, core_ids=list(range(8)),
                               trace=trace, **spmd_kwargs)
    out = np.empty((B, S, D), np.float32)
    for c in range(8):
        b, hq = c // 2, c % 2
        out[b, hq * SQ:(hq + 1) * SQ, :] = np.asarray(
            res.results[c]["outT"], np.float32).T
    out += bo.astype(np.float32)
    return out, res


def kernel(x, Wq, bq, Wk, bk, Wv, bv, Wo, bo):
    out, _ = run(np.asarray(x, np.float32), np.asarray(Wq, np.float32),
                 np.asarray(bq, np.float32), np.asarray(Wk, np.float32),
                 np.asarray(bk, np.float32), np.asarray(Wv, np.float32),
                 np.asarray(bv, np.float32), np.asarray(Wo, np.float32),
                 np.asarray(bo, np.float32))
    return out


# revision 17
# speedup vs baseline: 1.2387x; 1.2387x over previous
"""Multi-head attention (B=4, S=2048, D=1024, H=16) on 8 Trainium2 cores.

Sharding: core c -> (batch b=c//2, query-half hq=c%2). Each core computes
K/V projections for its batch's full sequence (no collectives needed) and
attention + output projection for its 1024 query rows.

Device dataflow (activations kept transposed, [feature, seq], except ctx):
  kT[e,s]    = WkT.T-contract  (lhsT=WkT[d,e] tiles, rhs=xT[d,s])
  qTz[h]     = per-head zero-padded q [128, SQ]: head h's 64 dims at
               partitions (h%2)*64, rest zero.  Scores then contract over
               the full K=128 partitions (K=64 matmuls stream at half rate
               on trn2; zero rows make K=128 exact and full speed).
  v[s,e]     = lhsT=xT[d,s] tiles, rhs=WvT[d,e]  (+bias via DVE add of a
               partition-broadcast bv)
  per (q-chunk qc of 512, head h, k-pair kh):
    scoresT[k,q] = kT.T-contract qTz  (2 matmuls/kh -> [128,1024] psum)
    expT = ScalarE Exp(scale=0.125) -> bf16 sbuf
    flipped ctx: for each 128-q subtile qi: ctq[q,hd+1] += expT-slice
               (stationary, M=128) @ vv[kt][:,h,:] (moving, N=65);
               col 64 = softmax denominator (ones column of vv)
    norm: reciprocal_approx_fast [128,1] + tensor_scalar_mul (per-q denom
               is per-PARTITION in this layout), PE-transpose back to
               ctxn[d, q]
  outT[e,q]  = WoT.T-contract ctxn  (bias bo added host-side)
Projections for e-tiles >= 1 and v-chunks >= 1 are emitted as PE "filler"
groups inside the attention loop so the PE works while ScalarE exps pace
the attention pipeline.
Host: out[b, hq*1024:(hq+1)*1024, :] = outT.T + bo
"""

import numpy as np
import ml_dtypes

import concourse.bacc as bacc
import concourse.tile as tile
from concourse import mybir
from concourse.bass_utils import run_bass_kernel_spmd
from concourse.masks import make_identity

B, S, D = 4, 2048, 1024
H, HD = 16, 64
SQ = 1024          # query rows per core
NDT = D // 128     # 8 d-tiles
NET = D // 128     # 8 e-tiles
NKT = S // 128     # 16 k-tiles
NST = S // 128     # 16 s-tiles
NQC = SQ // 512    # 2 q-chunks per core
BF16 = mybir.dt.bfloat16
F32 = mybir.dt.float32
SCALE = 1.0 / 8.0  # 1/sqrt(HD)

_NC_CACHE = None


def build_nc():
    nc = bacc.Bacc(None, target_bir_lowering=False, debug=True)

    xT_d = nc.declare_dram_parameter("xT", [D, S], BF16, isOutput=False)
    WqT_d = nc.declare_dram_parameter("WqT", [D, D], BF16, isOutput=False)
    WkT_d = nc.declare_dram_parameter("WkT", [D, D], BF16, isOutput=False)
    WvT_d = nc.declare_dram_parameter("WvT", [D, D], BF16, isOutput=False)
    WoT_d = nc.declare_dram_parameter("WoT", [D, D], BF16, isOutput=False)
    bqt_d = nc.declare_dram_parameter("bqt", [128, NET], F32, isOutput=False)
    bkt_d = nc.declare_dram_parameter("bkt", [128, NET], F32, isOutput=False)
    bvr_d = nc.declare_dram_parameter("bvr", [1, D], F32, isOutput=False)
    outT_d = nc.declare_dram_parameter("outT", [D, SQ], F32, isOutput=True)

    VC = 256           # v-projection chunk width (4 heads per chunk)
    NVC = D // VC      # 4 chunks

    with tile.TileContext(nc) as tc:
        with tc.tile_pool(name="resident", bufs=1) as res:
            # ---- resident SBUF tensors ----
            kT = [res.tile([128, S], BF16, name=f"kT{t}", tag=f"kT{t}")
                  for t in range(NET)]
            qTz = [res.tile([128, SQ], BF16, name=f"qTz{h}", tag=f"qTz{h}")
                   for h in range(H)]
            vv = [res.tile([128, H, HD + 1], BF16, name=f"v{t}", tag=f"v{t}")
                  for t in range(NST)]
            ctxn = [[res.tile([128, 512], BF16, name=f"ctxn{qc}_{t}",
                              tag=f"ctxn{qc}_{t}") for t in range(NDT)]
                    for qc in range(NQC)]
            Wo_t = [res.tile([128, D], BF16, name=f"Wo{t}", tag=f"Wo{t}")
                    for t in range(NDT)]
            xT = [res.tile([128, S], BF16, name=f"xT{t}", tag=f"xT{t}")
                  for t in range(NDT)]
            bq_dma = res.tile([128, NET], F32, tag="bq_dma")
            bk_dma = res.tile([128, NET], F32, tag="bk_dma")
            bq_sb = res.tile([128, NET], F32, tag="bq_sb")
            bk_sb = res.tile([128, NET], F32, tag="bk_sb")
            bv_sb = res.tile([1, D], F32, tag="bv_sb")
            bv_bc = res.tile([128, D], F32, tag="bv_bc")
            ident = res.tile([128, 128], BF16, tag="ident")

            nc.sync.dma_start(out=bq_dma, in_=bqt_d[:, :])
            nc.sync.dma_start(out=bk_dma, in_=bkt_d[:, :])
            nc.sync.dma_start(out=bv_sb, in_=bvr_d[:, :])
            # TensorScalarPtr has a single sync-wait slot; route the biases
            # through DVE once so later readers rely on program order.
            nc.vector.tensor_copy(out=bq_sb, in_=bq_dma)
            nc.vector.tensor_copy(out=bk_sb, in_=bk_dma)
            nc.gpsimd.partition_broadcast(bv_bc, bv_sb[0:1, :])
            make_identity(nc, ident)
            for h in range(H):
                z0 = 64 if h % 2 == 0 else 0
                nc.vector.memset(qTz[h][z0:z0 + 64, :], 0.0)
            for t in range(NST):
                # only the denominator column; cols 0:HD are overwritten
                nc.vector.memset(vv[t][:, :, HD:HD + 1], 1.0)

            # split x transfers so they spread across DMA queues and the
            # first projection group isn't gated on one big transfer
            for t in range(NDT):
                for c in range(4):
                    nc.sync.dma_start(
                        out=xT[t][:, c * 512:(c + 1) * 512],
                        in_=xT_d[t * 128:(t + 1) * 128, c * 512:(c + 1) * 512])
            for t in range(NDT):
                nc.sync.dma_start(out=Wo_t[t], in_=WoT_d[t * 128:(t + 1) * 128, :])

            with tc.tile_pool(name="p2", bufs=1) as p2:
                psum_src = {}

                def proj_ps():
                    return psum_src["pool"].tile(
                        [128, 512], F32, name="ps", tag=psum_src["tag"],
                        bufs=psum_src["bufs"])

                # ---------- projection emitters (also used as fillers) ----
                # weight SLICES are DMA-streamed per e-tile/chunk so that
                # three full weight sets never have to live in SBUF at once
                wk_cache = {}
                wq_cache = {}
                wv_cache = {}

                def w_slices(cache, key, W_d, c0, c1, tag, bufs):
                    if key not in cache:
                        ws = []
                        for dt in range(NDT):
                            wt = p2.tile([128, c1 - c0], BF16,
                                         name=f"{tag}{dt}", tag=tag,
                                         bufs=bufs)
                            nc.sync.dma_start(
                                out=wt, in_=W_d[dt * 128:(dt + 1) * 128,
                                                c0:c1])
                            ws.append(wt)
                        cache.clear()
                        cache[key] = ws
                    return cache[key]

                def emit_k_group(et, sc):
                    ws = w_slices(wk_cache, ("k", et), WkT_d,
                                  et * 128, (et + 1) * 128, "wks", 18)
                    ps = proj_ps()
                    for dt in range(NDT):
                        nc.tensor.matmul(
                            ps, ws[dt],
                            xT[dt][:, sc * 512: sc * 512 + 512],
                            start=(dt == 0), stop=(dt == NDT - 1))
                    nc.vector.tensor_scalar_add(
                        out=kT[et][:, sc * 512:(sc + 1) * 512],
                        in0=ps,
                        scalar1=bk_sb[:, et:et + 1])

                def emit_q_group(et, sc):
                    ws = w_slices(wq_cache, ("q", et), WqT_d,
                                  et * 128, (et + 1) * 128, "wqs", 18)
                    ps = proj_ps()
                    for dt in range(NDT):
                        nc.tensor.matmul(
                            ps, ws[dt],
                            xT[dt][:, sc * 512: sc * 512 + 512],
                            start=(dt == 0), stop=(dt == NDT - 1))
                    sl = slice(sc * 512, (sc + 1) * 512)
                    nc.vector.tensor_scalar_add(
                        out=qTz[2 * et][0:64, sl],
                        in0=ps[0:64, :],
                        scalar1=bq_sb[0:64, et:et + 1])
                    nc.vector.tensor_scalar_add(
                        out=qTz[2 * et + 1][64:128, sl],
                        in0=ps[64:128, :],
                        scalar1=bq_sb[64:128, et:et + 1])

                def emit_v_group(st, c):
                    # v chunk c covers e-columns [c*VC, (c+1)*VC) = 4 heads
                    ws = w_slices(wv_cache, ("v", c), WvT_d,
                                  c * VC, (c + 1) * VC, "wvs", 18)
                    psw = proj_ps()
                    ps = psw[:, 0:VC]
                    for dt in range(NDT):
                        nc.tensor.matmul(
                            ps,
                            xT[dt][:, st * 128:(st + 1) * 128],
                            ws[dt],
                            start=(dt == 0), stop=(dt == NDT - 1))
                    nh = VC // HD
                    nc.vector.tensor_add(
                        out=vv[st][:, c * nh:(c + 1) * nh, 0:HD],
                        in0=ps.rearrange("p (h d) -> p h d", h=nh),
                        in1=bv_bc[:, c * VC:(c + 1) * VC].rearrange(
                            "p (h d) -> p h d", h=nh))

                # ---------- upfront: first e-tile + first v chunk ----------
                with tc.psum_pool(name="pf", bufs=1) as pf:
                    psum_src.update(pool=pf, tag="pfg", bufs=2)
                    for sc in range(S // 512):
                        emit_k_group(0, sc)
                    for sc in range(NQC):
                        emit_q_group(0, sc)
                    for st in range(NST):
                        emit_v_group(st, 0)
                _cms = [tc.psum_pool(name="sp", bufs=2),
                        tc.psum_pool(name="cp", bufs=2),
                        tc.psum_pool(name="op", bufs=1),
                        tc.psum_pool(name="tp", bufs=1)]
                sp, cp, op, tp = [cm.__enter__() for cm in _cms]
                psum_src.update(pool=op, tag="op", bufs=1)

                fillers = []
                for et in range(1, NET):
                    for sc in range(S // 512):
                        fillers.append((emit_k_group, et, sc))
                    for sc in range(NQC):
                        fillers.append((emit_q_group, et, sc))
                    if et in (2, 4, 6):
                        c = et // 2
                        for st in range(NST):
                            fillers.append((emit_v_group, st, c))
                fillers = fillers[::-1]  # pop from the end

                # ---------- attention ----------
                def emit_sc(qc, h, kh):
                    ht = h // 2
                    sc_ps = sp.tile([128, 1024], F32, name="sc_ps",
                                    tag="sc", bufs=2)
                    for j in range(2):
                        kt = kh * 2 + j
                        nc.tensor.matmul(
                            sc_ps[:, j * 512:(j + 1) * 512],
                            kT[ht][:, kt * 128:(kt + 1) * 128],
                            qTz[h][:, qc * 512:(qc + 1) * 512],
                            start=True, stop=True)
                    return sc_ps

                def emit_outproj(qc_o, et):
                    ps = op.tile([128, 512], F32, name="ops", tag="op",
                                 bufs=1)
                    for dt in range(NDT):
                        nc.tensor.matmul(
                            ps,
                            Wo_t[dt][:, et * 128:(et + 1) * 128],
                            ctxn[qc_o][dt][:, :],
                            start=(dt == 0), stop=(dt == NDT - 1))
                    osb = p2.tile([128, 512], F32, name="osb", tag="osb",
                                  bufs=2)
                    nc.vector.tensor_copy(out=osb, in_=ps)
                    nc.sync.dma_start(
                        out=outT_d[et * 128:(et + 1) * 128,
                                   qc_o * 512:(qc_o + 1) * 512],
                        in_=osb)

                def emit_norm(ctq_ps, qi, qc, h):
                    # per-q denominators sit per-PARTITION in flipped layout
                    ht, hp = h // 2, (h % 2) * 64
                    inv = p2.tile([128, 1], F32, name="inv", tag="inv",
                                  bufs=3)
                    nc.vector.reciprocal_approx_fast(
                        inv, ctq_ps[:, qi, HD:HD + 1])
                    ctqn = p2.tile([128, HD], BF16, name="ctqn", tag="ctqn",
                                   bufs=3)
                    nc.vector.tensor_scalar_mul(
                        out=ctqn, in0=ctq_ps[:, qi, 0:HD], scalar1=inv)
                    tp_ps = tp.tile([HD, 128], BF16, name="tp_ps", tag="tp",
                                    bufs=1)
                    nc.tensor.transpose(tp_ps, ctqn, ident[:, :])
                    nc.vector.tensor_copy(
                        out=ctxn[qc][ht][hp:hp + HD, qi * 128:(qi + 1) * 128],
                        in_=tp_ps)

                iters = [(qc, h, kh)
                         for qc in range(NQC)
                         for h in range(H)
                         for kh in range(NKT // 2)]
                op_queue = []
                norm_queue = []
                ctq_state = {"ps": None}

                def emit_ctx_step(expT, qc, h, kh):
                    # flipped ctx: expT slice stationary (M=128 q), v moving
                    # (N=65); 4 q-subtiles accumulate in one psum bank
                    if kh == 0:
                        ctq_state["ps"] = cp.tile([128, 4, HD + 1], F32,
                                                  name="ctq", tag="ctq",
                                                  bufs=2)
                    ctq_ps = ctq_state["ps"]
                    for qi in range(4):
                        for j in range(2):
                            kt = kh * 2 + j
                            # start only on the bank's first write: the
                            # start bit marks the WHOLE 2KB bank pending-
                            # zero, so sibling qi regions must not re-set it
                            nc.tensor.matmul(
                                ctq_ps[:, qi, :],
                                expT[:, j * 512 + qi * 128:
                                     j * 512 + qi * 128 + 128],
                                vv[kt][:, h, :],
                                start=(kt == 0 and qi == 0),
                                stop=(kt == NKT - 1))
                    if kh == NKT // 2 - 1:
                        for qi in range(4):
                            norm_queue.append((ctq_ps, qi, qc, h))
                        if qc == 0 and h == H - 1:
                            op_queue.extend((0, et) for et in range(NET))

                sc_next = emit_sc(*iters[0])
                delayed = []
                for i, (qc, h, kh) in enumerate(iters):
                    sc_ps = sc_next
                    expT = p2.tile([128, 1024], BF16, name="expT",
                                   tag="expT", bufs=4)
                    nc.scalar.activation(
                        expT, sc_ps,
                        mybir.ActivationFunctionType.Exp,
                        scale=SCALE)
                    if i + 1 < len(iters):
                        sc_next = emit_sc(*iters[i + 1])
                    delayed.append((expT, qc, h, kh))
                    if len(delayed) > 1:
                        emit_ctx_step(*delayed.pop(0))
                    if norm_queue:
                        emit_norm(*norm_queue.pop(0))
                    if fillers:
                        fn, *args = fillers.pop()
                        fn(*args)
                    if kh == 6 and op_queue and h % 2 == 1:
                        emit_outproj(*op_queue.pop(0))
                while delayed:
                    emit_ctx_step(*delayed.pop(0))
                while norm_queue:
                    emit_norm(*norm_queue.pop(0))
                for args in op_queue:
                    emit_outproj(*args)
                for et in range(NET):
                    emit_outproj(1, et)
                for cm in reversed(_cms):
                    cm.__exit__(None, None, None)
    nc.compile()
    return nc


def _get_nc():
    global _NC_CACHE
    if _NC_CACHE is None:
        _NC_CACHE = build_nc()
    return _NC_CACHE


def _prep_maps(x, Wq, bq, Wk, bk, Wv, bv, Wo):
    bf = ml_dtypes.bfloat16
    WqT = np.ascontiguousarray(Wq.T).astype(bf)
    WkT = np.ascontiguousarray(Wk.T).astype(bf)
    WvT = np.ascontiguousarray(Wv.T).astype(bf)
    WoT = np.ascontiguousarray(Wo.T).astype(bf)
    bqt = np.ascontiguousarray(bq.reshape(NET, 128).T).astype(np.float32)
    bkt = np.ascontiguousarray(bk.reshape(NET, 128).T).astype(np.float32)
    bvr = np.ascontiguousarray(bv.reshape(1, D)).astype(np.float32)
    in_maps = []
    for c in range(8):
        b, hq = c // 2, c % 2
        xTb = np.ascontiguousarray(x[b].T).astype(bf)  # [D, S]
        if hq == 1:
            # rotate so local query half sits at columns [0, SQ)
            xTb = np.ascontiguousarray(
                np.concatenate([xTb[:, SQ:], xTb[:, :SQ]], axis=1))
        in_maps.append(dict(xT=xTb, WqT=WqT, WkT=WkT, WvT=WvT, WoT=WoT,
                            bqt=bqt, bkt=bkt, bvr=bvr))
    return in_maps


def run(x, Wq, bq, Wk, bk, Wv, bv, Wo, bo, trace=False, **spmd_kwargs):
    nc = _get_nc()
    in_maps = _prep_maps(x, Wq, bq, Wk, bk, Wv, bv, Wo)
    res = run_bass_kernel_spmd(nc, in_maps, core_ids=list(range(8)),
                               trace=trace, **spmd_kwargs)
    out = np.empty((B, S, D), np.float32)
    for c in range(8):
        b, hq = c // 2, c % 2
        out[b, hq * SQ:(hq + 1) * SQ, :] = np.asarray(
            res.results[c]["outT"], np.float32).T
    out += bo.astype(np.float32)
    return out, res


def kernel(x, Wq, bq, Wk, bk, Wv, bv, Wo, bo):
    out, _ = run(np.asarray(x, np.float32), np.asarray(Wq, np.float32),
                 np.asarray(bq, np.float32), np.asarray(Wk, np.float32),
                 np.asarray(bk, np.float32), np.asarray(Wv, np.float32),
                 np.asarray(bv, np.float32), np.asarray(Wo, np.float32),
                 np.asarray(bo, np.float32))
    return out


# revision 19
# speedup vs baseline: 1.2859x; 1.0381x over previous
"""Multi-head attention (B=4, S=2048, D=1024, H=16) on 8 Trainium2 cores.

Sharding: core c -> (batch b=c//2, query-half hq=c%2). Each core computes
K/V projections for its batch's full sequence (no collectives needed) and
attention + output projection for its 1024 query rows.

Device dataflow (activations kept transposed, [feature, seq], except ctx):
  kT[e,s]    = WkT.T-contract  (lhsT=WkT[d,e] tiles, rhs=xT[d,s])
  qTz[h]     = per-head zero-padded q [128, SQ]: head h's 64 dims at
               partitions (h%2)*64, rest zero.  Scores then contract over
               the full K=128 partitions (K=64 matmuls stream at half rate
               on trn2; zero rows make K=128 exact and full speed).
  v[s,e]     = lhsT=xT[d,s] tiles, rhs=WvT[d,e]  (+bias via DVE add of a
               partition-broadcast bv)
  per (q-chunk qc of 512, head h, k-pair kh):
    scoresT[k,q] = kT.T-contract qTz  (2 matmuls/kh -> [128,1024] psum)
    expT = ScalarE Exp(scale=0.125) -> bf16 sbuf
    flipped ctx: for each 128-q subtile qi: ctq[q,hd+1] += expT-slice
               (stationary, M=128) @ vv[kt][:,h,:] (moving, N=65);
               col 64 = softmax denominator (ones column of vv)
    norm: reciprocal_approx_fast [128,1] + tensor_scalar_mul (per-q denom
               is per-PARTITION in this layout), PE-transpose back to
               ctxn[d, q]
  outT[e,q]  = WoT.T-contract ctxn  (bias bo added host-side)
Projections for e-tiles >= 1 and v-chunks >= 1 are emitted as PE "filler"
groups inside the attention loop so the PE works while ScalarE exps pace
the attention pipeline.
Host: out[b, hq*1024:(hq+1)*1024, :] = outT.T + bo
"""

import numpy as np
import ml_dtypes

import concourse.bacc as bacc
import concourse.tile as tile
from concourse import mybir
from concourse.bass_utils import run_bass_kernel_spmd
from concourse.masks import make_identity

B, S, D = 4, 2048, 1024
H, HD = 16, 64
SQ = 1024          # query rows per core
NDT = D // 128     # 8 d-tiles
NET = D // 128     # 8 e-tiles
NKT = S // 128     # 16 k-tiles
NST = S // 128     # 16 s-tiles
NQC = SQ // 512    # 2 q-chunks per core
BF16 = mybir.dt.bfloat16
F32 = mybir.dt.float32
SCALE = 1.0 / 8.0  # 1/sqrt(HD)

_NC_CACHE = None


def build_nc():
    nc = bacc.Bacc(None, target_bir_lowering=False, debug=True)

    xT_d = nc.declare_dram_parameter("xT", [D, S], BF16, isOutput=False)
    WqT_d = nc.declare_dram_parameter("WqT", [D, D], BF16, isOutput=False)
    WkT_d = nc.declare_dram_parameter("WkT", [D, D], BF16, isOutput=False)
    WvT_d = nc.declare_dram_parameter("WvT", [D, D], BF16, isOutput=False)
    WoT_d = nc.declare_dram_parameter("WoT", [D, D], BF16, isOutput=False)
    bqt_d = nc.declare_dram_parameter("bqt", [128, NET], F32, isOutput=False)
    bkt_d = nc.declare_dram_parameter("bkt", [128, NET], F32, isOutput=False)
    bvr_d = nc.declare_dram_parameter("bvr", [1, D], F32, isOutput=False)
    outT_d = nc.declare_dram_parameter("outT", [D, SQ], F32, isOutput=True)

    VC = 256           # v-projection chunk width (4 heads per chunk)
    NVC = D // VC      # 4 chunks

    with tile.TileContext(nc) as tc:
        with tc.tile_pool(name="resident", bufs=1) as res:
            # ---- resident SBUF tensors ----
            kT = [res.tile([128, S], BF16, name=f"kT{t}", tag=f"kT{t}")
                  for t in range(NET)]
            qTz = [res.tile([128, SQ], BF16, name=f"qTz{h}", tag=f"qTz{h}")
                   for h in range(H)]
            vv = [res.tile([128, H, HD + 1], BF16, name=f"v{t}", tag=f"v{t}")
                  for t in range(NST)]
            ctxn = [[res.tile([128, 512], BF16, name=f"ctxn{qc}_{t}",
                              tag=f"ctxn{qc}_{t}") for t in range(NDT)]
                    for qc in range(NQC)]
            Wo_t = [res.tile([128, D], BF16, name=f"Wo{t}", tag=f"Wo{t}")
                    for t in range(NDT)]
            xT = [res.tile([128, S], BF16, name=f"xT{t}", tag=f"xT{t}")
                  for t in range(NDT)]
            bq_dma = res.tile([128, NET], F32, tag="bq_dma")
            bk_dma = res.tile([128, NET], F32, tag="bk_dma")
            bq_sb = res.tile([128, NET], F32, tag="bq_sb")
            bk_sb = res.tile([128, NET], F32, tag="bk_sb")
            bv_sb = res.tile([1, D], F32, tag="bv_sb")
            bv_bc = res.tile([128, D], F32, tag="bv_bc")
            ident = res.tile([128, 128], BF16, tag="ident")

            nc.sync.dma_start(out=bq_dma, in_=bqt_d[:, :])
            nc.sync.dma_start(out=bk_dma, in_=bkt_d[:, :])
            nc.sync.dma_start(out=bv_sb, in_=bvr_d[:, :])
            # TensorScalarPtr has a single sync-wait slot; route the biases
            # through DVE once so later readers rely on program order.
            nc.vector.tensor_copy(out=bq_sb, in_=bq_dma)
            nc.vector.tensor_copy(out=bk_sb, in_=bk_dma)
            nc.gpsimd.partition_broadcast(bv_bc, bv_sb[0:1, :])
            make_identity(nc, ident)
            for h in range(H):
                z0 = 64 if h % 2 == 0 else 0
                nc.vector.memset(qTz[h][z0:z0 + 64, :], 0.0)
            for t in range(NST):
                # only the denominator column; cols 0:HD are overwritten
                nc.vector.memset(vv[t][:, :, HD:HD + 1], 1.0)

            for t in range(NDT):
                nc.sync.dma_start(out=xT[t], in_=xT_d[t * 128:(t + 1) * 128, :])
            for t in range(NDT):
                nc.sync.dma_start(out=Wo_t[t], in_=WoT_d[t * 128:(t + 1) * 128, :])

            with tc.tile_pool(name="p2", bufs=1) as p2:
                psum_src = {}

                def proj_ps():
                    return psum_src["pool"].tile(
                        [128, 512], F32, name="ps", tag=psum_src["tag"],
                        bufs=psum_src["bufs"])

                # ---------- projection emitters (also used as fillers) ----
                # weight SLICES are DMA-streamed per e-tile/chunk so that
                # three full weight sets never have to live in SBUF at once
                wk_cache = {}
                wq_cache = {}
                wv_cache = {}

                def w_slices(cache, key, W_d, c0, c1, tag, bufs):
                    if key not in cache:
                        ws = []
                        for dt in range(NDT):
                            wt = p2.tile([128, c1 - c0], BF16,
                                         name=f"{tag}{dt}", tag=tag,
                                         bufs=bufs)
                            nc.sync.dma_start(
                                out=wt, in_=W_d[dt * 128:(dt + 1) * 128,
                                                c0:c1])
                            ws.append(wt)
                        cache.clear()
                        cache[key] = ws
                    return cache[key]

                def emit_k_group(et, sc):
                    ws = w_slices(wk_cache, ("k", et), WkT_d,
                                  et * 128, (et + 1) * 128, "wks", 18)
                    ps = proj_ps()
                    for dt in range(NDT):
                        nc.tensor.matmul(
                            ps, ws[dt],
                            xT[dt][:, sc * 512: sc * 512 + 512],
                            start=(dt == 0), stop=(dt == NDT - 1))
                    nc.vector.tensor_scalar_add(
                        out=kT[et][:, sc * 512:(sc + 1) * 512],
                        in0=ps,
                        scalar1=bk_sb[:, et:et + 1])

                def emit_q_group(et, sc):
                    ws = w_slices(wq_cache, ("q", et), WqT_d,
                                  et * 128, (et + 1) * 128, "wqs", 18)
                    ps = proj_ps()
                    for dt in range(NDT):
                        nc.tensor.matmul(
                            ps, ws[dt],
                            xT[dt][:, sc * 512: sc * 512 + 512],
                            start=(dt == 0), stop=(dt == NDT - 1))
                    sl = slice(sc * 512, (sc + 1) * 512)
                    nc.vector.tensor_scalar_add(
                        out=qTz[2 * et][0:64, sl],
                        in0=ps[0:64, :],
                        scalar1=bq_sb[0:64, et:et + 1])
                    nc.vector.tensor_scalar_add(
                        out=qTz[2 * et + 1][64:128, sl],
                        in0=ps[64:128, :],
                        scalar1=bq_sb[64:128, et:et + 1])

                def emit_v_group(st, c):
                    # v chunk c covers e-columns [c*VC, (c+1)*VC) = 4 heads
                    ws = w_slices(wv_cache, ("v", c), WvT_d,
                                  c * VC, (c + 1) * VC, "wvs", 18)
                    psw = proj_ps()
                    ps = psw[:, 0:VC]
                    for dt in range(NDT):
                        nc.tensor.matmul(
                            ps,
                            xT[dt][:, st * 128:(st + 1) * 128],
                            ws[dt],
                            start=(dt == 0), stop=(dt == NDT - 1))
                    nh = VC // HD
                    nc.vector.tensor_add(
                        out=vv[st][:, c * nh:(c + 1) * nh, 0:HD],
                        in0=ps.rearrange("p (h d) -> p h d", h=nh),
                        in1=bv_bc[:, c * VC:(c + 1) * VC].rearrange(
                            "p (h d) -> p h d", h=nh))

                # ---------- upfront: first e-tile + first v chunk ----------
                with tc.psum_pool(name="pf", bufs=1) as pf:
                    psum_src.update(pool=pf, tag="pfg", bufs=2)
                    for sc in range(S // 512):
                        emit_k_group(0, sc)
                    for sc in range(NQC):
                        emit_q_group(0, sc)
                    for st in range(NST):
                        emit_v_group(st, 0)
                _cms = [tc.psum_pool(name="sp", bufs=2),
                        tc.psum_pool(name="cp", bufs=2),
                        tc.psum_pool(name="op", bufs=1),
                        tc.psum_pool(name="tp", bufs=1)]
                sp, cp, op, tp = [cm.__enter__() for cm in _cms]
                psum_src.update(pool=op, tag="op", bufs=1)

                fillers = []
                for et in range(1, NET):
                    for sc in range(S // 512):
                        fillers.append((emit_k_group, et, sc))
                    for sc in range(NQC):
                        fillers.append((emit_q_group, et, sc))
                    if et in (2, 4, 6):
                        c = et // 2
                        for st in range(NST):
                            fillers.append((emit_v_group, st, c))
                fillers = fillers[::-1]  # pop from the end

                # ---------- attention ----------
                def emit_sc(qc, h, kh):
                    ht = h // 2
                    sc_ps = sp.tile([128, 1024], F32, name="sc_ps",
                                    tag="sc", bufs=2)
                    for j in range(2):
                        kt = kh * 2 + j
                        nc.tensor.matmul(
                            sc_ps[:, j * 512:(j + 1) * 512],
                            kT[ht][:, kt * 128:(kt + 1) * 128],
                            qTz[h][:, qc * 512:(qc + 1) * 512],
                            start=True, stop=True)
                    return sc_ps

                def emit_outproj(qc_o, et):
                    ps = op.tile([128, 512], F32, name="ops", tag="op",
                                 bufs=1)
                    for dt in range(NDT):
                        nc.tensor.matmul(
                            ps,
                            Wo_t[dt][:, et * 128:(et + 1) * 128],
                            ctxn[qc_o][dt][:, :],
                            start=(dt == 0), stop=(dt == NDT - 1))
                    osb = p2.tile([128, 512], F32, name="osb", tag="osb",
                                  bufs=2)
                    nc.vector.tensor_copy(out=osb, in_=ps)
                    nc.sync.dma_start(
                        out=outT_d[et * 128:(et + 1) * 128,
                                   qc_o * 512:(qc_o + 1) * 512],
                        in_=osb)

                def emit_norm(ctq_ps, qi, qc, h):
                    # per-q denominators sit per-PARTITION in flipped layout
                    ht, hp = h // 2, (h % 2) * 64
                    inv = p2.tile([128, 1], F32, name="inv", tag="inv",
                                  bufs=3)
                    nc.vector.reciprocal_approx_fast(
                        inv, ctq_ps[:, qi, HD:HD + 1])
                    ctqn = p2.tile([128, HD], BF16, name="ctqn", tag="ctqn",
                                   bufs=3)
                    nc.vector.tensor_scalar_mul(
                        out=ctqn, in0=ctq_ps[:, qi, 0:HD], scalar1=inv)
                    tp_ps = tp.tile([HD, 128], BF16, name="tp_ps", tag="tp",
                                    bufs=1)
                    nc.tensor.transpose(tp_ps, ctqn, ident[:, :])
                    nc.vector.tensor_copy(
                        out=ctxn[qc][ht][hp:hp + HD, qi * 128:(qi + 1) * 128],
                        in_=tp_ps)

                iters = [(qc, h, kh)
                         for qc in range(NQC)
                         for h in range(H)
                         for kh in range(NKT // 2)]
                op_queue = []
                norm_queue = []
                ctq_state = {"ps": None}

                def emit_ctx_step(expT, qc, h, kh):
                    # flipped ctx: expT slice stationary (M=128 q), v moving
                    # (N=65); 4 q-subtiles accumulate in one psum bank
                    if kh == 0:
                        ctq_state["ps"] = cp.tile([128, 4, HD + 1], F32,
                                                  name="ctq", tag="ctq",
                                                  bufs=2)
                    ctq_ps = ctq_state["ps"]
                    for qi in range(4):
                        for j in range(2):
                            kt = kh * 2 + j
                            # start only on the bank's first write: the
                            # start bit marks the WHOLE 2KB bank pending-
                            # zero, so sibling qi regions must not re-set it
                            nc.tensor.matmul(
                                ctq_ps[:, qi, :],
                                expT[:, j * 512 + qi * 128:
                                     j * 512 + qi * 128 + 128],
                                vv[kt][:, h, :],
                                start=(kt == 0 and qi == 0),
                                stop=(kt == NKT - 1))
                    if kh == NKT // 2 - 1:
                        for qi in range(4):
                            norm_queue.append((ctq_ps, qi, qc, h))
                        if qc == 0 and h == H - 1:
                            op_queue.extend((0, et) for et in range(NET))

                sc_next = emit_sc(*iters[0])
                delayed = []
                for i, (qc, h, kh) in enumerate(iters):
                    sc_ps = sc_next
                    expT = p2.tile([128, 1024], BF16, name="expT",
                                   tag="expT", bufs=6)
                    nc.scalar.activation(
                        expT, sc_ps,
                        mybir.ActivationFunctionType.Exp,
                        scale=SCALE)
                    if i + 1 < len(iters):
                        sc_next = emit_sc(*iters[i + 1])
                    delayed.append((expT, qc, h, kh))
                    if len(delayed) > 1:
                        emit_ctx_step(*delayed.pop(0))
                    if norm_queue:
                        emit_norm(*norm_queue.pop(0))
                    for _ in range(2 if i < 16 else 1):
                        if fillers:
                            fn, *args = fillers.pop()
                            fn(*args)
                    if kh == 6 and op_queue and h % 2 == 1:
                        emit_outproj(*op_queue.pop(0))
                while delayed:
                    emit_ctx_step(*delayed.pop(0))
                while norm_queue:
                    emit_norm(*norm_queue.pop(0))
                for args in op_queue:
                    emit_outproj(*args)
                for et in range(NET):
                    emit_outproj(1, et)
                for cm in reversed(_cms):
                    cm.__exit__(None, None, None)
    nc.compile()
    return nc


def _get_nc():
    global _NC_CACHE
    if _NC_CACHE is None:
        _NC_CACHE = build_nc()
    return _NC_CACHE


def _prep_maps(x, Wq, bq, Wk, bk, Wv, bv, Wo):
    bf = ml_dtypes.bfloat16
    WqT = np.ascontiguousarray(Wq.T).astype(bf)
    WkT = np.ascontiguousarray(Wk.T).astype(bf)
    WvT = np.ascontiguousarray(Wv.T).astype(bf)
    WoT = np.ascontiguousarray(Wo.T).astype(bf)
    bqt = np.ascontiguousarray(bq.reshape(NET, 128).T).astype(np.float32)
    bkt = np.ascontiguousarray(bk.reshape(NET, 128).T).astype(np.float32)
    bvr = np.ascontiguousarray(bv.reshape(1, D)).astype(np.float32)
    in_maps = []
    for c in range(8):
        b, hq = c // 2, c % 2
        xTb = np.ascontiguousarray(x[b].T).astype(bf)  # [D, S]
        if hq == 1:
            # rotate so local query half sits at columns [0, SQ)
            xTb = np.ascontiguousarray(
                np.concatenate([xTb[:, SQ:], xTb[:, :SQ]], axis=1))
        in_maps.append(dict(xT=xTb, WqT=WqT, WkT=WkT, WvT=WvT, WoT=WoT,
                            bqt=bqt, bkt=bkt, bvr=bvr))
    return in_maps


def run(x, Wq, bq, Wk, bk, Wv, bv, Wo, bo, trace=False, **spmd_kwargs):
    nc = _get_nc()
    in_maps = _prep_maps(x, Wq, bq, Wk, bk, Wv, bv, Wo)
    res = run_bass_kernel_spmd(nc, in_maps, core_ids=list(range(8)),
                               trace=trace, **spmd_kwargs)
    out = np.empty((B, S, D), np.float32)
    for c in range(8):
        b, hq = c // 2, c % 2
        out[b, hq * SQ:(hq + 1) * SQ, :] = np.asarray(
            res.results[c]["outT"], np.float32).T
    out += bo.astype(np.float32)
    return out, res


def kernel(x, Wq, bq, Wk, bk, Wv, bv, Wo, bo):
    out, _ = run(np.asarray(x, np.float32), np.asarray(Wq, np.float32),
                 np.asarray(bq, np.float32), np.asarray(Wk, np.float32),
                 np.asarray(bk, np.float32), np.asarray(Wv, np.float32),
                 np.asarray(bv, np.float32), np.asarray(Wo, np.float32),
                 np.asarray(bo, np.float32))
    return out


# revision 20
# speedup vs baseline: 1.3107x; 1.0193x over previous
"""Multi-head attention (B=4, S=2048, D=1024, H=16) on 8 Trainium2 cores.

Sharding: core c -> (batch b=c//2, query-half hq=c%2). Each core computes
K/V projections for its batch's full sequence (no collectives needed) and
attention + output projection for its 1024 query rows.

Device dataflow (activations kept transposed, [feature, seq], except ctx):
  kT[e,s]    = WkT.T-contract  (lhsT=WkT[d,e] tiles, rhs=xT[d,s])
  qTz[h]     = per-head zero-padded q [128, SQ]: head h's 64 dims at
               partitions (h%2)*64, rest zero.  Scores then contract over
               the full K=128 partitions (K=64 matmuls stream at half rate
               on trn2; zero rows make K=128 exact and full speed).
  v[s,e]     = lhsT=xT[d,s] tiles, rhs=WvT[d,e]  (+bias via DVE add of a
               partition-broadcast bv)
  per (q-chunk qc of 512, head h, k-pair kh):
    scoresT[k,q] = kT.T-contract qTz  (2 matmuls/kh -> [128,1024] psum)
    expT = ScalarE Exp(scale=0.125) -> bf16 sbuf
    flipped ctx: for each 128-q subtile qi: ctq[q,hd+1] += expT-slice
               (stationary, M=128) @ vv[kt][:,h,:] (moving, N=65);
               col 64 = softmax denominator (ones column of vv)
    norm: reciprocal_approx_fast [128,1] + tensor_scalar_mul (per-q denom
               is per-PARTITION in this layout), PE-transpose back to
               ctxn[d, q]
  outT[e,q]  = WoT.T-contract ctxn  (bias bo added host-side)
Projections for e-tiles >= 1 and v-chunks >= 1 are emitted as PE "filler"
groups inside the attention loop so the PE works while ScalarE exps pace
the attention pipeline.
Host: out[b, hq*1024:(hq+1)*1024, :] = outT.T + bo
"""

import numpy as np
import ml_dtypes

import concourse.bacc as bacc
import concourse.tile as tile
from concourse import mybir
from concourse.bass_utils import run_bass_kernel_spmd
from concourse.masks import make_identity

B, S, D = 4, 2048, 1024
H, HD = 16, 64
SQ = 1024          # query rows per core
NDT = D // 128     # 8 d-tiles
NET = D // 128     # 8 e-tiles
NKT = S // 128     # 16 k-tiles
NST = S // 128     # 16 s-tiles
NQC = SQ // 512    # 2 q-chunks per core
BF16 = mybir.dt.bfloat16
F32 = mybir.dt.float32
SCALE = 1.0 / 8.0  # 1/sqrt(HD)

_NC_CACHE = None


def build_nc():
    nc = bacc.Bacc(None, target_bir_lowering=False, debug=True)

    xT_d = nc.declare_dram_parameter("xT", [D, S], BF16, isOutput=False)
    WqT_d = nc.declare_dram_parameter("WqT", [D, D], BF16, isOutput=False)
    WkT_d = nc.declare_dram_parameter("WkT", [D, D], BF16, isOutput=False)
    WvT_d = nc.declare_dram_parameter("WvT", [D, D], BF16, isOutput=False)
    WoT_d = nc.declare_dram_parameter("WoT", [D, D], BF16, isOutput=False)
    bqt_d = nc.declare_dram_parameter("bqt", [128, NET], F32, isOutput=False)
    bkt_d = nc.declare_dram_parameter("bkt", [128, NET], F32, isOutput=False)
    bvr_d = nc.declare_dram_parameter("bvr", [1, D], F32, isOutput=False)
    outT_d = nc.declare_dram_parameter("outT", [D, SQ], F32, isOutput=True)

    VC = 256           # v-projection chunk width (4 heads per chunk)
    NVC = D // VC      # 4 chunks

    with tile.TileContext(nc) as tc:
        with tc.tile_pool(name="resident", bufs=1) as res:
            # ---- resident SBUF tensors ----
            kT = [res.tile([128, S], BF16, name=f"kT{t}", tag=f"kT{t}")
                  for t in range(NET)]
            qTz = [res.tile([128, SQ], BF16, name=f"qTz{h}", tag=f"qTz{h}")
                   for h in range(H)]
            vv = [res.tile([128, H, HD + 1], BF16, name=f"v{t}", tag=f"v{t}")
                  for t in range(NST)]
            ctxn = [[res.tile([128, 512], BF16, name=f"ctxn{qc}_{t}",
                              tag=f"ctxn{qc}_{t}") for t in range(NDT)]
                    for qc in range(NQC)]
            Wo_t = [res.tile([128, D], BF16, name=f"Wo{t}", tag=f"Wo{t}")
                    for t in range(NDT)]
            xT = [res.tile([128, S], BF16, name=f"xT{t}", tag=f"xT{t}")
                  for t in range(NDT)]
            bq_dma = res.tile([128, NET], F32, tag="bq_dma")
            bk_dma = res.tile([128, NET], F32, tag="bk_dma")
            bq_sb = res.tile([128, NET], F32, tag="bq_sb")
            bk_sb = res.tile([128, NET], F32, tag="bk_sb")
            bv_sb = res.tile([1, D], F32, tag="bv_sb")
            bv_bc = res.tile([128, D], F32, tag="bv_bc")
            ident = res.tile([128, 128], BF16, tag="ident")

            nc.sync.dma_start(out=bq_dma, in_=bqt_d[:, :])
            nc.sync.dma_start(out=bk_dma, in_=bkt_d[:, :])
            nc.sync.dma_start(out=bv_sb, in_=bvr_d[:, :])
            # TensorScalarPtr has a single sync-wait slot; route the biases
            # through DVE once so later readers rely on program order.
            nc.vector.tensor_copy(out=bq_sb, in_=bq_dma)
            nc.vector.tensor_copy(out=bk_sb, in_=bk_dma)
            nc.gpsimd.partition_broadcast(bv_bc, bv_sb[0:1, :])
            make_identity(nc, ident)
            for h in range(H):
                z0 = 64 if h % 2 == 0 else 0
                nc.vector.memset(qTz[h][z0:z0 + 64, :], 0.0)
            for t in range(NST):
                # only the denominator column; cols 0:HD are overwritten
                nc.vector.memset(vv[t][:, :, HD:HD + 1], 1.0)

            for t in range(NDT):
                nc.sync.dma_start(out=xT[t], in_=xT_d[t * 128:(t + 1) * 128, :])
            for t in range(NDT):
                nc.sync.dma_start(out=Wo_t[t], in_=WoT_d[t * 128:(t + 1) * 128, :])

            with tc.tile_pool(name="p2", bufs=1) as p2:
                psum_src = {}

                def proj_ps():
                    return psum_src["pool"].tile(
                        [128, 512], F32, name="ps", tag=psum_src["tag"],
                        bufs=psum_src["bufs"])

                # ---------- projection emitters (also used as fillers) ----
                # weight SLICES are DMA-streamed per e-tile/chunk so that
                # three full weight sets never have to live in SBUF at once
                wk_cache = {}
                wq_cache = {}
                wv_cache = {}

                def w_slices(cache, key, W_d, c0, c1, tag, bufs):
                    if key not in cache:
                        ws = []
                        for dt in range(NDT):
                            wt = p2.tile([128, c1 - c0], BF16,
                                         name=f"{tag}{dt}", tag=tag,
                                         bufs=bufs)
                            nc.sync.dma_start(
                                out=wt, in_=W_d[dt * 128:(dt + 1) * 128,
                                                c0:c1])
                            ws.append(wt)
                        cache.clear()
                        cache[key] = ws
                    return cache[key]

                def emit_k_group(et, sc):
                    ws = w_slices(wk_cache, ("k", et), WkT_d,
                                  et * 128, (et + 1) * 128, "wks", 18)
                    ps = proj_ps()
                    for dt in range(NDT):
                        nc.tensor.matmul(
                            ps, ws[dt],
                            xT[dt][:, sc * 512: sc * 512 + 512],
                            start=(dt == 0), stop=(dt == NDT - 1))
                    nc.vector.tensor_scalar_add(
                        out=kT[et][:, sc * 512:(sc + 1) * 512],
                        in0=ps,
                        scalar1=bk_sb[:, et:et + 1])

                def emit_q_group(et, sc):
                    ws = w_slices(wq_cache, ("q", et), WqT_d,
                                  et * 128, (et + 1) * 128, "wqs", 18)
                    ps = proj_ps()
                    for dt in range(NDT):
                        nc.tensor.matmul(
                            ps, ws[dt],
                            xT[dt][:, sc * 512: sc * 512 + 512],
                            start=(dt == 0), stop=(dt == NDT - 1))
                    sl = slice(sc * 512, (sc + 1) * 512)
                    nc.vector.tensor_scalar_add(
                        out=qTz[2 * et][0:64, sl],
                        in0=ps[0:64, :],
                        scalar1=bq_sb[0:64, et:et + 1])
                    nc.vector.tensor_scalar_add(
                        out=qTz[2 * et + 1][64:128, sl],
                        in0=ps[64:128, :],
                        scalar1=bq_sb[64:128, et:et + 1])

                def emit_v_group(st, c):
                    # v chunk c covers e-columns [c*VC, (c+1)*VC) = 4 heads
                    ws = w_slices(wv_cache, ("v", c), WvT_d,
                                  c * VC, (c + 1) * VC, "wvs", 18)
                    psw = proj_ps()
                    ps = psw[:, 0:VC]
                    for dt in range(NDT):
                        nc.tensor.matmul(
                            ps,
                            xT[dt][:, st * 128:(st + 1) * 128],
                            ws[dt],
                            start=(dt == 0), stop=(dt == NDT - 1))
                    nh = VC // HD
                    nc.vector.tensor_add(
                        out=vv[st][:, c * nh:(c + 1) * nh, 0:HD],
                        in0=ps.rearrange("p (h d) -> p h d", h=nh),
                        in1=bv_bc[:, c * VC:(c + 1) * VC].rearrange(
                            "p (h d) -> p h d", h=nh))

                # ---------- upfront: first e-tile + first v chunk ----------
                with tc.psum_pool(name="pf", bufs=1) as pf:
                    psum_src.update(pool=pf, tag="pfg", bufs=2)
                    for sc in range(S // 512):
                        emit_k_group(0, sc)
                    for sc in range(NQC):
                        emit_q_group(0, sc)
                _cms = [tc.psum_pool(name="sp", bufs=2),
                        tc.psum_pool(name="cp", bufs=2),
                        tc.psum_pool(name="op", bufs=1),
                        tc.psum_pool(name="tp", bufs=1)]
                sp, cp, op, tp = [cm.__enter__() for cm in _cms]
                psum_src.update(pool=op, tag="op", bufs=2)

                fillers = [(emit_v_group, st, 0) for st in range(NST)]
                for et in range(1, NET):
                    for sc in range(S // 512):
                        fillers.append((emit_k_group, et, sc))
                    for sc in range(NQC):
                        fillers.append((emit_q_group, et, sc))
                    if et in (2, 4, 6):
                        c = et // 2
                        for st in range(NST):
                            fillers.append((emit_v_group, st, c))
                fillers = fillers[::-1]  # pop from the end

                # ---------- attention ----------
                def emit_sc(qc, h, kh):
                    ht = h // 2
                    sc_ps = sp.tile([128, 1024], F32, name="sc_ps",
                                    tag="sc", bufs=2)
                    for j in range(2):
                        kt = kh * 2 + j
                        nc.tensor.matmul(
                            sc_ps[:, j * 512:(j + 1) * 512],
                            kT[ht][:, kt * 128:(kt + 1) * 128],
                            qTz[h][:, qc * 512:(qc + 1) * 512],
                            start=True, stop=True)
                    return sc_ps

                def emit_outproj(qc_o, et):
                    ps = op.tile([128, 512], F32, name="ops", tag="op",
                                 bufs=2)
                    for dt in range(NDT):
                        nc.tensor.matmul(
                            ps,
                            Wo_t[dt][:, et * 128:(et + 1) * 128],
                            ctxn[qc_o][dt][:, :],
                            start=(dt == 0), stop=(dt == NDT - 1))
                    osb = p2.tile([128, 512], F32, name="osb", tag="osb",
                                  bufs=2)
                    nc.vector.tensor_copy(out=osb, in_=ps)
                    nc.sync.dma_start(
                        out=outT_d[et * 128:(et + 1) * 128,
                                   qc_o * 512:(qc_o + 1) * 512],
                        in_=osb)

                def emit_norm(ctq_ps, qi, qc, h):
                    # per-q denominators sit per-PARTITION in flipped layout
                    ht, hp = h // 2, (h % 2) * 64
                    inv = p2.tile([128, 1], F32, name="inv", tag="inv",
                                  bufs=3)
                    nc.vector.reciprocal_approx_fast(
                        inv, ctq_ps[:, qi, HD:HD + 1])
                    ctqn = p2.tile([128, HD], BF16, name="ctqn", tag="ctqn",
                                   bufs=3)
                    nc.vector.tensor_scalar_mul(
                        out=ctqn, in0=ctq_ps[:, qi, 0:HD], scalar1=inv)
                    tp_ps = tp.tile([HD, 128], BF16, name="tp_ps", tag="tp",
                                    bufs=1)
                    nc.tensor.transpose(tp_ps, ctqn, ident[:, :])
                    nc.vector.tensor_copy(
                        out=ctxn[qc][ht][hp:hp + HD, qi * 128:(qi + 1) * 128],
                        in_=tp_ps)

                iters = [(qc, h, kh)
                         for qc in range(NQC)
                         for h in range(H)
                         for kh in range(NKT // 2)]
                op_queue = []
                norm_queue = []
                ctq_state = {"ps": None}

                def emit_ctx_step(expT, qc, h, kh):
                    # flipped ctx: expT slice stationary (M=128 q), v moving
                    # (N=65); 4 q-subtiles accumulate in one psum bank
                    if kh == 0:
                        ctq_state["ps"] = cp.tile([128, 4, HD + 1], F32,
                                                  name="ctq", tag="ctq",
                                                  bufs=1)
                    ctq_ps = ctq_state["ps"]
                    for qi in range(4):
                        for j in range(2):
                            kt = kh * 2 + j
                            # start only on the bank's first write: the
                            # start bit marks the WHOLE 2KB bank pending-
                            # zero, so sibling qi regions must not re-set it
                            nc.tensor.matmul(
                                ctq_ps[:, qi, :],
                                expT[:, j * 512 + qi * 128:
                                     j * 512 + qi * 128 + 128],
                                vv[kt][:, h, :],
                                start=(kt == 0 and qi == 0),
                                stop=(kt == NKT - 1))
                    if kh == NKT // 2 - 1:
                        ctq_sb = p2.tile([128, 4, HD + 1], F32,
                                         name="ctq_sb", tag="ctq_sb",
                                         bufs=2)
                        nc.vector.tensor_copy(out=ctq_sb, in_=ctq_ps)
                        for qi in range(4):
                            norm_queue.append((ctq_sb, qi, qc, h))
                        if qc == 0 and h == H - 1:
                            op_queue.extend((0, et) for et in range(NET))

                sc_next = emit_sc(*iters[0])
                delayed = []
                for i, (qc, h, kh) in enumerate(iters):
                    sc_ps = sc_next
                    expT = p2.tile([128, 1024], BF16, name="expT",
                                   tag="expT", bufs=6)
                    nc.scalar.activation(
                        expT, sc_ps,
                        mybir.ActivationFunctionType.Exp,
                        scale=SCALE)
                    if i + 1 < len(iters):
                        sc_next = emit_sc(*iters[i + 1])
                    delayed.append((expT, qc, h, kh))
                    if len(delayed) > 1:
                        emit_ctx_step(*delayed.pop(0))
                    if norm_queue:
                        emit_norm(*norm_queue.pop(0))
                    for _ in range(2 if i < 16 else 1):
                        if fillers:
                            fn, *args = fillers.pop()
                            fn(*args)
                    if kh == 6 and op_queue and h % 2 == 1:
                        emit_outproj(*op_queue.pop(0))
                while delayed:
                    emit_ctx_step(*delayed.pop(0))
                while norm_queue:
                    emit_norm(*norm_queue.pop(0))
                for args in op_queue:
                    emit_outproj(*args)
                for et in range(NET):
                    emit_outproj(1, et)
                for cm in reversed(_cms):
                    cm.__exit__(None, None, None)
    nc.compile()
    return nc


def _get_nc():
    global _NC_CACHE
    if _NC_CACHE is None:
        _NC_CACHE = build_nc()
    return _NC_CACHE


def _prep_maps(x, Wq, bq, Wk, bk, Wv, bv, Wo):
    bf = ml_dtypes.bfloat16
    WqT = np.ascontiguousarray(Wq.T).astype(bf)
    WkT = np.ascontiguousarray(Wk.T).astype(bf)
    WvT = np.ascontiguousarray(Wv.T).astype(bf)
    WoT = np.ascontiguousarray(Wo.T).astype(bf)
    bqt = np.ascontiguousarray(bq.reshape(NET, 128).T).astype(np.float32)
    bkt = np.ascontiguousarray(bk.reshape(NET, 128).T).astype(np.float32)
    bvr = np.ascontiguousarray(bv.reshape(1, D)).astype(np.float32)
    in_maps = []
    for c in range(8):
        b, hq = c // 2, c % 2
        xTb = np.ascontiguousarray(x[b].T).astype(bf)  # [D, S]
        if hq == 1:
            # rotate so local query half sits at columns [0, SQ)
            xTb = np.ascontiguousarray(
                np.concatenate([xTb[:, SQ:], xTb[:, :SQ]], axis=1))
        in_maps.append(dict(xT=xTb, WqT=WqT, WkT=WkT, WvT=WvT, WoT=WoT,
                            bqt=bqt, bkt=bkt, bvr=bvr))
    return in_maps


def run(x, Wq, bq, Wk, bk, Wv, bv, Wo, bo, trace=False, **spmd_kwargs):
    nc = _get_nc()
    in_maps = _prep_maps(x, Wq, bq, Wk, bk, Wv, bv, Wo)
    res = run_bass_kernel_spmd(nc, in_maps, core_ids=list(range(8)),
                               trace=trace, **spmd_kwargs)
    out = np.empty((B, S, D), np.float32)
    for c in range(8):
        b, hq = c // 2, c % 2
        out[b, hq * SQ:(hq + 1) * SQ, :] = np.asarray(
            res.results[c]["outT"], np.float32).T
    out += bo.astype(np.float32)
    return out, res


def kernel(x, Wq, bq, Wk, bk, Wv, bv, Wo, bo):
    out, _ = run(np.asarray(x, np.float32), np.asarray(Wq, np.float32),
                 np.asarray(bq, np.float32), np.asarray(Wk, np.float32),
                 np.asarray(bk, np.float32), np.asarray(Wv, np.float32),
                 np.asarray(bv, np.float32), np.asarray(Wo, np.float32),
                 np.asarray(bo, np.float32))
    return out


# revision 21
# speedup vs baseline: 1.3266x; 1.0121x over previous
"""Multi-head attention (B=4, S=2048, D=1024, H=16) on 8 Trainium2 cores.

Sharding: core c -> (batch b=c//2, query-half hq=c%2). Each core computes
K/V projections for its batch's full sequence (no collectives needed) and
attention + output projection for its 1024 query rows.

Device dataflow (activations kept transposed, [feature, seq], except ctx):
  kT[e,s]    = WkT.T-contract  (lhsT=WkT[d,e] tiles, rhs=xT[d,s])
  qTz[h]     = per-head zero-padded q [128, SQ]: head h's 64 dims at
               partitions (h%2)*64, rest zero.  Scores then contract over
               the full K=128 partitions (K=64 matmuls stream at half rate
               on trn2; zero rows make K=128 exact and full speed).
  v[s,e]     = lhsT=xT[d,s] tiles, rhs=WvT[d,e]  (+bias via DVE add of a
               partition-broadcast bv)
  per (q-chunk qc of 512, head h, k-pair kh):
    scoresT[k,q] = kT.T-contract qTz  (2 matmuls/kh -> [128,1024] psum)
    expT = ScalarE Exp(scale=0.125) -> bf16 sbuf
    flipped ctx: for each 128-q subtile qi: ctq[q,hd+1] += expT-slice
               (stationary, M=128) @ vv[kt][:,h,:] (moving, N=65);
               col 64 = softmax denominator (ones column of vv)
    norm: reciprocal_approx_fast [128,1] + tensor_scalar_mul (per-q denom
               is per-PARTITION in this layout), PE-transpose back to
               ctxn[d, q]
  outT[e,q]  = WoT.T-contract ctxn  (bias bo added host-side)
Projections for e-tiles >= 1 and v-chunks >= 1 are emitted as PE "filler"
groups inside the attention loop so the PE works while ScalarE exps pace
the attention pipeline.
Host: out[b, hq*1024:(hq+1)*1024, :] = outT.T + bo
"""

import numpy as np
import ml_dtypes

import concourse.bacc as bacc
import concourse.tile as tile
from concourse import mybir
from concourse.bass_utils import run_bass_kernel_spmd
from concourse.masks import make_identity

B, S, D = 4, 2048, 1024
H, HD = 16, 64
SQ = 1024          # query rows per core
NDT = D // 128     # 8 d-tiles
NET = D // 128     # 8 e-tiles
NKT = S // 128     # 16 k-tiles
NST = S // 128     # 16 s-tiles
NQC = SQ // 512    # 2 q-chunks per core
BF16 = mybir.dt.bfloat16
F32 = mybir.dt.float32
SCALE = 1.0 / 8.0  # 1/sqrt(HD)

_NC_CACHE = None


def build_nc():
    nc = bacc.Bacc(None, target_bir_lowering=False, debug=True)

    xT_d = nc.declare_dram_parameter("xT", [D, S], BF16, isOutput=False)
    WqT_d = nc.declare_dram_parameter("WqT", [D, D], BF16, isOutput=False)
    WkT_d = nc.declare_dram_parameter("WkT", [D, D], BF16, isOutput=False)
    WvT_d = nc.declare_dram_parameter("WvT", [D, D], BF16, isOutput=False)
    WoT_d = nc.declare_dram_parameter("WoT", [D, D], BF16, isOutput=False)
    bqt_d = nc.declare_dram_parameter("bqt", [128, NET], F32, isOutput=False)
    bkt_d = nc.declare_dram_parameter("bkt", [128, NET], F32, isOutput=False)
    bvr_d = nc.declare_dram_parameter("bvr", [1, D], F32, isOutput=False)
    outT_d = nc.declare_dram_parameter("outT", [D, SQ], F32, isOutput=True)

    VC = 256           # v-projection chunk width (4 heads per chunk)
    NVC = D // VC      # 4 chunks

    with tile.TileContext(nc) as tc:
        with tc.tile_pool(name="resident", bufs=1) as res:
            # ---- resident SBUF tensors ----
            kT = [res.tile([128, S], BF16, name=f"kT{t}", tag=f"kT{t}")
                  for t in range(NET)]
            qTz = [res.tile([128, SQ], BF16, name=f"qTz{h}", tag=f"qTz{h}")
                   for h in range(H)]
            vv = [res.tile([128, H, HD + 1], BF16, name=f"v{t}", tag=f"v{t}")
                  for t in range(NST)]
            ctxn = [[res.tile([128, 512], BF16, name=f"ctxn{qc}_{t}",
                              tag=f"ctxn{qc}_{t}") for t in range(NDT)]
                    for qc in range(NQC)]
            Wo_t = [res.tile([128, D], BF16, name=f"Wo{t}", tag=f"Wo{t}")
                    for t in range(NDT)]
            xT = [res.tile([128, S], BF16, name=f"xT{t}", tag=f"xT{t}")
                  for t in range(NDT)]
            bq_dma = res.tile([128, NET], F32, tag="bq_dma")
            bk_dma = res.tile([128, NET], F32, tag="bk_dma")
            bq_sb = res.tile([128, NET], F32, tag="bq_sb")
            bk_sb = res.tile([128, NET], F32, tag="bk_sb")
            bv_sb = res.tile([1, D], F32, tag="bv_sb")
            bv_bc = res.tile([128, D], F32, tag="bv_bc")
            ident = res.tile([128, 128], BF16, tag="ident")

            nc.sync.dma_start(out=bq_dma, in_=bqt_d[:, :])
            nc.sync.dma_start(out=bk_dma, in_=bkt_d[:, :])
            nc.sync.dma_start(out=bv_sb, in_=bvr_d[:, :])
            # TensorScalarPtr has a single sync-wait slot; route the biases
            # through DVE once so later readers rely on program order.
            nc.vector.tensor_copy(out=bq_sb, in_=bq_dma)
            nc.vector.tensor_copy(out=bk_sb, in_=bk_dma)
            nc.gpsimd.partition_broadcast(bv_bc, bv_sb[0:1, :])
            make_identity(nc, ident)
            for h in range(H):
                z0 = 64 if h % 2 == 0 else 0
                nc.vector.memset(qTz[h][z0:z0 + 64, :], 0.0)
            for t in range(NST):
                # only the denominator column; cols 0:HD are overwritten
                nc.vector.memset(vv[t][:, :, HD:HD + 1], 1.0)

            for t in range(NDT):
                nc.sync.dma_start(out=xT[t], in_=xT_d[t * 128:(t + 1) * 128, :])
            for t in range(NDT):
                nc.sync.dma_start(out=Wo_t[t], in_=WoT_d[t * 128:(t + 1) * 128, :])

            with tc.tile_pool(name="p2", bufs=1) as p2:
                psum_src = {}

                def proj_ps():
                    return psum_src["pool"].tile(
                        [128, 512], F32, name="ps", tag=psum_src["tag"],
                        bufs=psum_src["bufs"])

                # ---------- projection emitters (also used as fillers) ----
                # weight SLICES are DMA-streamed per e-tile/chunk so that
                # three full weight sets never have to live in SBUF at once
                wk_cache = {}
                wq_cache = {}
                wv_cache = {}

                def w_slices(cache, key, W_d, c0, c1, tag, bufs):
                    if key not in cache:
                        ws = []
                        for dt in range(NDT):
                            wt = p2.tile([128, c1 - c0], BF16,
                                         name=f"{tag}{dt}", tag=tag,
                                         bufs=bufs)
                            nc.sync.dma_start(
                                out=wt, in_=W_d[dt * 128:(dt + 1) * 128,
                                                c0:c1])
                            ws.append(wt)
                        cache.clear()
                        cache[key] = ws
                    return cache[key]

                def emit_k_group(et, sc):
                    ws = w_slices(wk_cache, ("k", et), WkT_d,
                                  et * 128, (et + 1) * 128, "wks", 18)
                    ps = proj_ps()
                    for dt in range(NDT):
                        nc.tensor.matmul(
                            ps, ws[dt],
                            xT[dt][:, sc * 512: sc * 512 + 512],
                            start=(dt == 0), stop=(dt == NDT - 1))
                    nc.vector.tensor_scalar_add(
                        out=kT[et][:, sc * 512:(sc + 1) * 512],
                        in0=ps,
                        scalar1=bk_sb[:, et:et + 1])

                def emit_q_group(et, sc):
                    ws = w_slices(wq_cache, ("q", et), WqT_d,
                                  et * 128, (et + 1) * 128, "wqs", 18)
                    ps = proj_ps()
                    for dt in range(NDT):
                        nc.tensor.matmul(
                            ps, ws[dt],
                            xT[dt][:, sc * 512: sc * 512 + 512],
                            start=(dt == 0), stop=(dt == NDT - 1))
                    sl = slice(sc * 512, (sc + 1) * 512)
                    nc.vector.tensor_scalar_add(
                        out=qTz[2 * et][0:64, sl],
                        in0=ps[0:64, :],
                        scalar1=bq_sb[0:64, et:et + 1])
                    nc.vector.tensor_scalar_add(
                        out=qTz[2 * et + 1][64:128, sl],
                        in0=ps[64:128, :],
                        scalar1=bq_sb[64:128, et:et + 1])

                def emit_v_group(st, c):
                    # v chunk c covers e-columns [c*VC, (c+1)*VC) = 4 heads
                    ws = w_slices(wv_cache, ("v", c), WvT_d,
                                  c * VC, (c + 1) * VC, "wvs", 18)
                    psw = proj_ps()
                    ps = psw[:, 0:VC]
                    for dt in range(NDT):
                        nc.tensor.matmul(
                            ps,
                            xT[dt][:, st * 128:(st + 1) * 128],
                            ws[dt],
                            start=(dt == 0), stop=(dt == NDT - 1))
                    nh = VC // HD
                    nc.vector.tensor_add(
                        out=vv[st][:, c * nh:(c + 1) * nh, 0:HD],
                        in0=ps.rearrange("p (h d) -> p h d", h=nh),
                        in1=bv_bc[:, c * VC:(c + 1) * VC].rearrange(
                            "p (h d) -> p h d", h=nh))

                # ---------- upfront: first e-tile + first v chunk ----------
                with tc.psum_pool(name="pf", bufs=1) as pf:
                    psum_src.update(pool=pf, tag="pfg", bufs=2)
                    for sc in range(S // 512):
                        emit_k_group(0, sc)
                    emit_q_group(0, 0)
                _cms = [tc.psum_pool(name="sp", bufs=2),
                        tc.psum_pool(name="cp", bufs=2),
                        tc.psum_pool(name="op", bufs=1),
                        tc.psum_pool(name="tp", bufs=1)]
                sp, cp, op, tp = [cm.__enter__() for cm in _cms]
                psum_src.update(pool=op, tag="op", bufs=2)

                fillers = [(emit_v_group, st, 0) for st in range(NST)]
                for et in range(1, NET):
                    for sc in range(S // 512):
                        fillers.append((emit_k_group, et, sc))
                    fillers.append((emit_q_group, et, 0))
                    if et in (2, 4, 6):
                        c = et // 2
                        for st in range(NST):
                            fillers.append((emit_v_group, st, c))
                fillers = fillers[::-1]  # pop from the end
                # q projections for the second q-chunk aren't consumed until
                # qc1 (iteration 128+16*et): drip them into the post-filler
                # bubble where ScalarE paces and the PE has slack
                late_fillers = [(emit_q_group, et, 1) for et in range(NET)]
                late_fillers = late_fillers[::-1]

                # ---------- attention ----------
                def emit_sc(qc, h, kh):
                    ht = h // 2
                    sc_ps = sp.tile([128, 1024], F32, name="sc_ps",
                                    tag="sc", bufs=2)
                    for j in range(2):
                        kt = kh * 2 + j
                        nc.tensor.matmul(
                            sc_ps[:, j * 512:(j + 1) * 512],
                            kT[ht][:, kt * 128:(kt + 1) * 128],
                            qTz[h][:, qc * 512:(qc + 1) * 512],
                            start=True, stop=True)
                    return sc_ps

                def emit_outproj(qc_o, et):
                    ps = op.tile([128, 512], F32, name="ops", tag="op",
                                 bufs=2)
                    for dt in range(NDT):
                        nc.tensor.matmul(
                            ps,
                            Wo_t[dt][:, et * 128:(et + 1) * 128],
                            ctxn[qc_o][dt][:, :],
                            start=(dt == 0), stop=(dt == NDT - 1))
                    osb = p2.tile([128, 512], F32, name="osb", tag="osb",
                                  bufs=2)
                    nc.vector.tensor_copy(out=osb, in_=ps)
                    nc.sync.dma_start(
                        out=outT_d[et * 128:(et + 1) * 128,
                                   qc_o * 512:(qc_o + 1) * 512],
                        in_=osb)

                def emit_norm(ctq_ps, qi, qc, h):
                    # per-q denominators sit per-PARTITION in flipped layout
                    ht, hp = h // 2, (h % 2) * 64
                    inv = p2.tile([128, 1], F32, name="inv", tag="inv",
                                  bufs=3)
                    nc.vector.reciprocal_approx_fast(
                        inv, ctq_ps[:, qi, HD:HD + 1])
                    ctqn = p2.tile([128, HD], BF16, name="ctqn", tag="ctqn",
                                   bufs=3)
                    nc.vector.tensor_scalar_mul(
                        out=ctqn, in0=ctq_ps[:, qi, 0:HD], scalar1=inv)
                    tp_ps = tp.tile([HD, 128], BF16, name="tp_ps", tag="tp",
                                    bufs=1)
                    nc.tensor.transpose(tp_ps, ctqn, ident[:, :])
                    nc.vector.tensor_copy(
                        out=ctxn[qc][ht][hp:hp + HD, qi * 128:(qi + 1) * 128],
                        in_=tp_ps)

                iters = [(qc, h, kh)
                         for qc in range(NQC)
                         for h in range(H)
                         for kh in range(NKT // 2)]
                op_queue = []
                norm_queue = []
                ctq_state = {"ps": None}

                def emit_ctx_step(expT, qc, h, kh):
                    # flipped ctx: expT slice stationary (M=128 q), v moving
                    # (N=65); 4 q-subtiles accumulate in one psum bank
                    if kh == 0:
                        ctq_state["ps"] = cp.tile([128, 4, HD + 1], F32,
                                                  name="ctq", tag="ctq",
                                                  bufs=1)
                    ctq_ps = ctq_state["ps"]
                    for qi in range(4):
                        for j in range(2):
                            kt = kh * 2 + j
                            # start only on the bank's first write: the
                            # start bit marks the WHOLE 2KB bank pending-
                            # zero, so sibling qi regions must not re-set it
                            nc.tensor.matmul(
                                ctq_ps[:, qi, :],
                                expT[:, j * 512 + qi * 128:
                                     j * 512 + qi * 128 + 128],
                                vv[kt][:, h, :],
                                start=(kt == 0 and qi == 0),
                                stop=(kt == NKT - 1))
                    if kh == NKT // 2 - 1:
                        ctq_sb = p2.tile([128, 4, HD + 1], F32,
                                         name="ctq_sb", tag="ctq_sb",
                                         bufs=2)
                        nc.vector.tensor_copy(out=ctq_sb, in_=ctq_ps)
                        for qi in range(4):
                            norm_queue.append((ctq_sb, qi, qc, h))
                        if qc == 0 and h == H - 1:
                            op_queue.extend((0, et) for et in range(NET))

                sc_next = emit_sc(*iters[0])
                delayed = []
                for i, (qc, h, kh) in enumerate(iters):
                    sc_ps = sc_next
                    expT = p2.tile([128, 1024], BF16, name="expT",
                                   tag="expT", bufs=6)
                    nc.scalar.activation(
                        expT, sc_ps,
                        mybir.ActivationFunctionType.Exp)
                    if i + 1 < len(iters):
                        sc_next = emit_sc(*iters[i + 1])
                    delayed.append((expT, qc, h, kh))
                    if len(delayed) > 1:
                        emit_ctx_step(*delayed.pop(0))
                    if norm_queue:
                        emit_norm(*norm_queue.pop(0))
                    for _ in range(2 if i < 16 else 1):
                        if fillers:
                            fn, *args = fillers.pop()
                            fn(*args)
                    if i >= 96 and i % 16 == 0 and late_fillers:
                        fn, *args = late_fillers.pop()
                        fn(*args)
                    if kh == 6 and op_queue and h % 2 == 1:
                        emit_outproj(*op_queue.pop(0))
                while delayed:
                    emit_ctx_step(*delayed.pop(0))
                while norm_queue:
                    emit_norm(*norm_queue.pop(0))
                for args in op_queue:
                    emit_outproj(*args)
                for et in range(NET):
                    emit_outproj(1, et)
                for cm in reversed(_cms):
                    cm.__exit__(None, None, None)
    nc.compile()
    return nc


def _get_nc():
    global _NC_CACHE
    if _NC_CACHE is None:
        _NC_CACHE = build_nc()
    return _NC_CACHE


def _prep_maps(x, Wq, bq, Wk, bk, Wv, bv, Wo):
    bf = ml_dtypes.bfloat16
    WqT = np.ascontiguousarray(Wq.T * SCALE).astype(bf)
    WkT = np.ascontiguousarray(Wk.T).astype(bf)
    WvT = np.ascontiguousarray(Wv.T).astype(bf)
    WoT = np.ascontiguousarray(Wo.T).astype(bf)
    bqt = np.ascontiguousarray(
        bq.reshape(NET, 128).T * SCALE).astype(np.float32)
    bkt = np.ascontiguousarray(bk.reshape(NET, 128).T).astype(np.float32)
    bvr = np.ascontiguousarray(bv.reshape(1, D)).astype(np.float32)
    in_maps = []
    for c in range(8):
        b, hq = c // 2, c % 2
        xTb = np.ascontiguousarray(x[b].T).astype(bf)  # [D, S]
        if hq == 1:
            # rotate so local query half sits at columns [0, SQ)
            xTb = np.ascontiguousarray(
                np.concatenate([xTb[:, SQ:], xTb[:, :SQ]], axis=1))
        in_maps.append(dict(xT=xTb, WqT=WqT, WkT=WkT, WvT=WvT, WoT=WoT,
                            bqt=bqt, bkt=bkt, bvr=bvr))
    return in_maps


def run(x, Wq, bq, Wk, bk, Wv, bv, Wo, bo, trace=False, **spmd_kwargs):
    nc = _get_nc()
    in_maps = _prep_maps(x, Wq, bq, Wk, bk, Wv, bv, Wo)
    res = run_bass_kernel_spmd(nc, in_maps, core_ids=list(range(8)),
                               trace=trace, **spmd_kwargs)
    out = np.empty((B, S, D), np.float32)
    for c in range(8):
        b, hq = c // 2, c % 2
        out[b, hq * SQ:(hq + 1) * SQ, :] = np.asarray(
            res.results[c]["outT"], np.float32).T
    out += bo.astype(np.float32)
    return out, res


def kernel(x, Wq, bq, Wk, bk, Wv, bv, Wo, bo):
    out, _ = run(np.asarray(x, np.float32), np.asarray(Wq, np.float32),
                 np.asarray(bq, np.float32), np.asarray(Wk, np.float32),
                 np.asarray(bk, np.float32), np.asarray(Wv, np.float32),
                 np.asarray(bv, np.float32), np.asarray(Wo, np.float32),
                 np.asarray(bo, np.float32))
    return out


# revision 23
# speedup vs baseline: 1.3348x; 1.0062x over previous
"""Multi-head attention (B=4, S=2048, D=1024, H=16) on 8 Trainium2 cores.

Sharding: core c -> (batch b=c//2, query-half hq=c%2). Each core computes
K/V projections for its batch's full sequence (no collectives needed) and
attention + output projection for its 1024 query rows.

Device dataflow (activations kept transposed, [feature, seq], except ctx):
  kT[e,s]    = WkT.T-contract  (lhsT=WkT[d,e] tiles, rhs=xT[d,s])
  qTz[h]     = per-head zero-padded q [128, SQ]: head h's 64 dims at
               partitions (h%2)*64, rest zero.  Scores then contract over
               the full K=128 partitions (K=64 matmuls stream at half rate
               on trn2; zero rows make K=128 exact and full speed).
  v[s,e]     = lhsT=xT[d,s] tiles, rhs=WvT[d,e]  (+bias via DVE add of a
               partition-broadcast bv)
  per (q-chunk qc of 512, head h, k-pair kh):
    scoresT[k,q] = kT.T-contract qTz  (2 matmuls/kh -> [128,1024] psum)
    expT = ScalarE Exp(scale=0.125) -> bf16 sbuf
    flipped ctx: for each 128-q subtile qi: ctq[q,hd+1] += expT-slice
               (stationary, M=128) @ vv[kt][:,h,:] (moving, N=65);
               col 64 = softmax denominator (ones column of vv)
    norm: reciprocal_approx_fast [128,1] + tensor_scalar_mul (per-q denom
               is per-PARTITION in this layout), PE-transpose back to
               ctxn[d, q]
  outT[e,q]  = WoT.T-contract ctxn  (bias bo added host-side)
Projections for e-tiles >= 1 and v-chunks >= 1 are emitted as PE "filler"
groups inside the attention loop so the PE works while ScalarE exps pace
the attention pipeline.
Host: out[b, hq*1024:(hq+1)*1024, :] = outT.T + bo
"""

import numpy as np
import ml_dtypes

import concourse.bacc as bacc
import concourse.tile as tile
from concourse import mybir
from concourse.bass_utils import run_bass_kernel_spmd
from concourse.masks import make_identity

B, S, D = 4, 2048, 1024
H, HD = 16, 64
SQ = 1024          # query rows per core
NDT = D // 128     # 8 d-tiles
NET = D // 128     # 8 e-tiles
NKT = S // 128     # 16 k-tiles
NST = S // 128     # 16 s-tiles
NQC = SQ // 512    # 2 q-chunks per core
BF16 = mybir.dt.bfloat16
F32 = mybir.dt.float32
SCALE = 1.0 / 8.0  # 1/sqrt(HD)

_NC_CACHE = None


def build_nc():
    nc = bacc.Bacc(None, target_bir_lowering=False, debug=True)

    xT_d = nc.declare_dram_parameter("xT", [D, S], BF16, isOutput=False)
    WqT_d = nc.declare_dram_parameter("WqT", [D, D], BF16, isOutput=False)
    WkT_d = nc.declare_dram_parameter("WkT", [D, D], BF16, isOutput=False)
    WvT_d = nc.declare_dram_parameter("WvT", [D, D], BF16, isOutput=False)
    WoT_d = nc.declare_dram_parameter("WoT", [D, D], BF16, isOutput=False)
    bqt_d = nc.declare_dram_parameter("bqt", [128, NET], F32, isOutput=False)
    bkt_d = nc.declare_dram_parameter("bkt", [128, NET], F32, isOutput=False)
    bvr_d = nc.declare_dram_parameter("bvr", [1, D], F32, isOutput=False)
    outT_d = nc.declare_dram_parameter("outT", [D, SQ], F32, isOutput=True)

    VC = 256           # v-projection chunk width (4 heads per chunk)
    NVC = D // VC      # 4 chunks

    with tile.TileContext(nc) as tc:
        with tc.tile_pool(name="resident", bufs=1) as res:
            # ---- resident SBUF tensors ----
            kT = [res.tile([128, S], BF16, name=f"kT{t}", tag=f"kT{t}")
                  for t in range(NET)]
            qTz = [res.tile([128, SQ], BF16, name=f"qTz{h}", tag=f"qTz{h}")
                   for h in range(H)]
            vv = [res.tile([128, H, HD + 1], BF16, name=f"v{t}", tag=f"v{t}")
                  for t in range(NST)]
            ctxn = [[res.tile([128, 512], BF16, name=f"ctxn{qc}_{t}",
                              tag=f"ctxn{qc}_{t}") for t in range(NDT)]
                    for qc in range(NQC)]
            Wo_t = [res.tile([128, D], BF16, name=f"Wo{t}", tag=f"Wo{t}")
                    for t in range(NDT)]
            xT = [res.tile([128, S], BF16, name=f"xT{t}", tag=f"xT{t}")
                  for t in range(NDT)]
            bq_dma = res.tile([128, NET], F32, tag="bq_dma")
            bk_dma = res.tile([128, NET], F32, tag="bk_dma")
            bq_sb = res.tile([128, NET], F32, tag="bq_sb")
            bk_sb = res.tile([128, NET], F32, tag="bk_sb")
            bv_sb = res.tile([1, D], F32, tag="bv_sb")
            bv_bc = res.tile([128, D], F32, tag="bv_bc")
            ident = res.tile([128, 128], BF16, tag="ident")

            nc.sync.dma_start(out=bq_dma, in_=bqt_d[:, :])
            nc.sync.dma_start(out=bk_dma, in_=bkt_d[:, :])
            nc.sync.dma_start(out=bv_sb, in_=bvr_d[:, :])
            # TensorScalarPtr has a single sync-wait slot; route the biases
            # through DVE once so later readers rely on program order.
            nc.vector.tensor_copy(out=bq_sb, in_=bq_dma)
            nc.vector.tensor_copy(out=bk_sb, in_=bk_dma)
            nc.gpsimd.partition_broadcast(bv_bc, bv_sb[0:1, :])
            make_identity(nc, ident)
            for h in range(H):
                z0 = 64 if h % 2 == 0 else 0
                nc.vector.memset(qTz[h][z0:z0 + 64, :], 0.0)
            for t in range(NST):
                # only the denominator column; cols 0:HD are overwritten
                nc.vector.memset(vv[t][:, :, HD:HD + 1], 1.0)

            for t in range(NDT):
                nc.sync.dma_start(out=xT[t], in_=xT_d[t * 128:(t + 1) * 128, :])
            for t in range(NDT):
                nc.sync.dma_start(out=Wo_t[t], in_=WoT_d[t * 128:(t + 1) * 128, :])

            with tc.tile_pool(name="p2", bufs=1) as p2:
                psum_src = {}

                def proj_ps():
                    return psum_src["pool"].tile(
                        [128, 512], F32, name="ps", tag=psum_src["tag"],
                        bufs=psum_src["bufs"])

                # ---------- projection emitters (also used as fillers) ----
                # weight SLICES are DMA-streamed per e-tile/chunk so that
                # three full weight sets never have to live in SBUF at once
                wk_cache = {}
                wq_cache = {}
                wv_cache = {}

                def w_slices(cache, key, W_d, c0, c1, tag, bufs):
                    if key not in cache:
                        ws = []
                        for dt in range(NDT):
                            wt = p2.tile([128, c1 - c0], BF16,
                                         name=f"{tag}{dt}", tag=tag,
                                         bufs=bufs)
                            nc.sync.dma_start(
                                out=wt, in_=W_d[dt * 128:(dt + 1) * 128,
                                                c0:c1])
                            ws.append(wt)
                        cache.clear()
                        cache[key] = ws
                    return cache[key]

                def emit_k_group(et, sc):
                    ws = w_slices(wk_cache, ("k", et), WkT_d,
                                  et * 128, (et + 1) * 128, "wks", 18)
                    ps = proj_ps()
                    for dt in range(NDT):
                        nc.tensor.matmul(
                            ps, ws[dt],
                            xT[dt][:, sc * 512: sc * 512 + 512],
                            start=(dt == 0), stop=(dt == NDT - 1))
                    nc.vector.tensor_scalar_add(
                        out=kT[et][:, sc * 512:(sc + 1) * 512],
                        in0=ps,
                        scalar1=bk_sb[:, et:et + 1])

                def emit_q_group(et, sc):
                    ws = w_slices(wq_cache, ("q", et), WqT_d,
                                  et * 128, (et + 1) * 128, "wqs", 18)
                    ps = proj_ps()
                    for dt in range(NDT):
                        nc.tensor.matmul(
                            ps, ws[dt],
                            xT[dt][:, sc * 512: sc * 512 + 512],
                            start=(dt == 0), stop=(dt == NDT - 1))
                    sl = slice(sc * 512, (sc + 1) * 512)
                    nc.vector.tensor_scalar_add(
                        out=qTz[2 * et][0:64, sl],
                        in0=ps[0:64, :],
                        scalar1=bq_sb[0:64, et:et + 1])
                    nc.vector.tensor_scalar_add(
                        out=qTz[2 * et + 1][64:128, sl],
                        in0=ps[64:128, :],
                        scalar1=bq_sb[64:128, et:et + 1])

                def emit_v_group(st, c):
                    # v chunk c covers e-columns [c*VC, (c+1)*VC) = 4 heads
                    ws = w_slices(wv_cache, ("v", c), WvT_d,
                                  c * VC, (c + 1) * VC, "wvs", 18)
                    psw = proj_ps()
                    ps = psw[:, 0:VC]
                    for dt in range(NDT):
                        nc.tensor.matmul(
                            ps,
                            xT[dt][:, st * 128:(st + 1) * 128],
                            ws[dt],
                            start=(dt == 0), stop=(dt == NDT - 1))
                    nh = VC // HD
                    nc.vector.tensor_add(
                        out=vv[st][:, c * nh:(c + 1) * nh, 0:HD],
                        in0=ps.rearrange("p (h d) -> p h d", h=nh),
                        in1=bv_bc[:, c * VC:(c + 1) * VC].rearrange(
                            "p (h d) -> p h d", h=nh))

                # ---------- upfront: first e-tile + first v chunk ----------
                with tc.psum_pool(name="pf", bufs=1) as pf:
                    psum_src.update(pool=pf, tag="pfg", bufs=2)
                    for sc in range(S // 512):
                        emit_k_group(0, sc)
                    emit_q_group(0, 0)
                _cms = [tc.psum_pool(name="sp", bufs=2),
                        tc.psum_pool(name="cp", bufs=2),
                        tc.psum_pool(name="op", bufs=1),
                        tc.psum_pool(name="tp", bufs=1)]
                sp, cp, op, tp = [cm.__enter__() for cm in _cms]
                psum_src.update(pool=op, tag="op", bufs=2)

                fillers = [(emit_v_group, st, 0) for st in range(NST)]
                for et in range(1, NET):
                    for sc in range(S // 512):
                        fillers.append((emit_k_group, et, sc))
                    fillers.append((emit_q_group, et, 0))
                    if et in (2, 4, 6):
                        c = et // 2
                        for st in range(NST):
                            fillers.append((emit_v_group, st, c))
                fillers = fillers[::-1]  # pop from the end
                # q projections for the second q-chunk aren't consumed until
                # qc1 (iteration 128+16*et): drip them into the post-filler
                # bubble where ScalarE paces and the PE has slack
                late_fillers = [(emit_q_group, et, 1) for et in range(NET)]
                late_fillers = late_fillers[::-1]

                # ---------- attention ----------
                def emit_sc(qc, h, kh):
                    ht = h // 2
                    sc_ps = sp.tile([128, 1024], F32, name="sc_ps",
                                    tag="sc", bufs=2)
                    for j in range(2):
                        kt = kh * 2 + j
                        nc.tensor.matmul(
                            sc_ps[:, j * 512:(j + 1) * 512],
                            kT[ht][:, kt * 128:(kt + 1) * 128],
                            qTz[h][:, qc * 512:(qc + 1) * 512],
                            start=True, stop=True)
                    return sc_ps

                def emit_outproj(qc_o, et):
                    ps = op.tile([128, 512], F32, name="ops", tag="op",
                                 bufs=2)
                    for dt in range(NDT):
                        nc.tensor.matmul(
                            ps,
                            Wo_t[dt][:, et * 128:(et + 1) * 128],
                            ctxn[qc_o][dt][:, :],
                            start=(dt == 0), stop=(dt == NDT - 1))
                    osb = p2.tile([128, 512], F32, name="osb", tag="osb",
                                  bufs=2)
                    nc.vector.tensor_copy(out=osb, in_=ps)
                    nc.gpsimd.dma_start(
                        out=outT_d[et * 128:(et + 1) * 128,
                                   qc_o * 512:(qc_o + 1) * 512],
                        in_=osb)

                def emit_norm(ctq_ps, qi, qc, h):
                    # per-q denominators sit per-PARTITION in flipped layout
                    ht, hp = h // 2, (h % 2) * 64
                    inv = p2.tile([128, 1], F32, name="inv", tag="inv",
                                  bufs=3)
                    nc.vector.reciprocal_approx_fast(
                        inv, ctq_ps[:, qi, HD:HD + 1])
                    ctqn = p2.tile([128, HD], BF16, name="ctqn", tag="ctqn",
                                   bufs=3)
                    nc.vector.tensor_scalar_mul(
                        out=ctqn, in0=ctq_ps[:, qi, 0:HD], scalar1=inv)
                    tp_ps = tp.tile([HD, 128], BF16, name="tp_ps", tag="tp",
                                    bufs=1)
                    nc.tensor.transpose(tp_ps, ctqn, ident[:, :])
                    nc.vector.tensor_copy(
                        out=ctxn[qc][ht][hp:hp + HD, qi * 128:(qi + 1) * 128],
                        in_=tp_ps)

                iters = [(qc, h, kh)
                         for qc in range(NQC)
                         for h in range(H)
                         for kh in range(NKT // 2)]
                op_queue = []
                norm_queue = []
                ctq_state = {"ps": None}

                def emit_ctx_step(expT, qc, h, kh):
                    # flipped ctx: expT slice stationary (M=128 q), v moving
                    # (N=65); 4 q-subtiles accumulate in one psum bank
                    if kh == 0:
                        ctq_state["ps"] = cp.tile([128, 4, HD + 1], F32,
                                                  name="ctq", tag="ctq",
                                                  bufs=1)
                    ctq_ps = ctq_state["ps"]
                    for qi in range(4):
                        for j in range(2):
                            kt = kh * 2 + j
                            # start only on the bank's first write: the
                            # start bit marks the WHOLE 2KB bank pending-
                            # zero, so sibling qi regions must not re-set it
                            nc.tensor.matmul(
                                ctq_ps[:, qi, :],
                                expT[:, j * 512 + qi * 128:
                                     j * 512 + qi * 128 + 128],
                                vv[kt][:, h, :],
                                start=(kt == 0 and qi == 0),
                                stop=(kt == NKT - 1))
                    if kh == NKT // 2 - 1:
                        ctq_sb = p2.tile([128, 4, HD + 1], F32,
                                         name="ctq_sb", tag="ctq_sb",
                                         bufs=2)
                        nc.vector.tensor_copy(out=ctq_sb, in_=ctq_ps)
                        for qi in range(4):
                            norm_queue.append((ctq_sb, qi, qc, h))
                        if qc == 0 and h == H - 1:
                            op_queue.extend((0, et) for et in range(NET))

                sc_next = emit_sc(*iters[0])
                delayed = []
                for i, (qc, h, kh) in enumerate(iters):
                    sc_ps = sc_next
                    expT = p2.tile([128, 1024], BF16, name="expT",
                                   tag="expT", bufs=6)
                    nc.scalar.activation(
                        expT, sc_ps,
                        mybir.ActivationFunctionType.Exp)
                    if i + 1 < len(iters):
                        sc_next = emit_sc(*iters[i + 1])
                    delayed.append((expT, qc, h, kh))
                    if len(delayed) > 1:
                        emit_ctx_step(*delayed.pop(0))
                    if norm_queue:
                        emit_norm(*norm_queue.pop(0))
                    for _ in range(2 if i < 16 else 1):
                        if fillers:
                            fn, *args = fillers.pop()
                            fn(*args)
                    if i >= 96 and i % 16 == 0 and late_fillers:
                        fn, *args = late_fillers.pop()
                        fn(*args)
                    if kh == 6 and op_queue and h % 2 == 1:
                        emit_outproj(*op_queue.pop(0))
                while delayed:
                    emit_ctx_step(*delayed.pop(0))
                while norm_queue:
                    emit_norm(*norm_queue.pop(0))
                for args in op_queue:
                    emit_outproj(*args)
                for et in range(NET):
                    emit_outproj(1, et)
                for cm in reversed(_cms):
                    cm.__exit__(None, None, None)
    nc.compile()
    return nc


def _get_nc():
    global _NC_CACHE
    if _NC_CACHE is None:
        _NC_CACHE = build_nc()
    return _NC_CACHE


def _prep_maps(x, Wq, bq, Wk, bk, Wv, bv, Wo):
    bf = ml_dtypes.bfloat16
    WqT = np.ascontiguousarray(Wq.T * SCALE).astype(bf)
    WkT = np.ascontiguousarray(Wk.T).astype(bf)
    WvT = np.ascontiguousarray(Wv.T).astype(bf)
    WoT = np.ascontiguousarray(Wo.T).astype(bf)
    bqt = np.ascontiguousarray(
        bq.reshape(NET, 128).T * SCALE).astype(np.float32)
    bkt = np.ascontiguousarray(bk.reshape(NET, 128).T).astype(np.float32)
    bvr = np.ascontiguousarray(bv.reshape(1, D)).astype(np.float32)
    in_maps = []
    for c in range(8):
        b, hq = c // 2, c % 2
        xTb = np.ascontiguousarray(x[b].T).astype(bf)  # [D, S]
        if hq == 1:
            # rotate so local query half sits at columns [0, SQ)
            xTb = np.ascontiguousarray(
                np.concatenate([xTb[:, SQ:], xTb[:, :SQ]], axis=1))
        in_maps.append(dict(xT=xTb, WqT=WqT, WkT=WkT, WvT=WvT, WoT=WoT,
                            bqt=bqt, bkt=bkt, bvr=bvr))
    return in_maps


def run(x, Wq, bq, Wk, bk, Wv, bv, Wo, bo, trace=False, **spmd_kwargs):
    nc = _get_nc()
    in_maps = _prep_maps(x, Wq, bq, Wk, bk, Wv, bv, Wo)
    res = run_bass_kernel_spmd(nc, in_maps, core_ids=list(range(8)),
                               trace=trace, **spmd_kwargs)
    out = np.empty((B, S, D), np.float32)
    for c in range(8):
        b, hq = c // 2, c % 2
        out[b, hq * SQ:(hq + 1) * SQ, :] = np.asarray(
            res.results[c]["outT"], np.float32).T
    out += bo.astype(np.float32)
    return out, res


def kernel(x, Wq, bq, Wk, bk, Wv, bv, Wo, bo):
    out, _ = run(np.asarray(x, np.float32), np.asarray(Wq, np.float32),
                 np.asarray(bq, np.float32), np.asarray(Wk, np.float32),
                 np.asarray(bk, np.float32), np.asarray(Wv, np.float32),
                 np.asarray(bv, np.float32), np.asarray(Wo, np.float32),
                 np.asarray(bo, np.float32))
    return out


# revision 24
# speedup vs baseline: 1.3365x; 1.0012x over previous
"""Multi-head attention (B=4, S=2048, D=1024, H=16) on 8 Trainium2 cores.

Sharding: core c -> (batch b=c//2, query-half hq=c%2). Each core computes
K/V projections for its batch's full sequence (no collectives needed) and
attention + output projection for its 1024 query rows.

Device dataflow (activations kept transposed, [feature, seq], except ctx):
  kT[e,s]    = WkT.T-contract  (lhsT=WkT[d,e] tiles, rhs=xT[d,s])
  qTz[h]     = per-head zero-padded q [128, SQ]: head h's 64 dims at
               partitions (h%2)*64, rest zero.  Scores then contract over
               the full K=128 partitions (K=64 matmuls stream at half rate
               on trn2; zero rows make K=128 exact and full speed).
  v[s,e]     = lhsT=xT[d,s] tiles, rhs=WvT[d,e]  (+bias via DVE add of a
               partition-broadcast bv)
  per (q-chunk qc of 512, head h, k-pair kh):
    scoresT[k,q] = kT.T-contract qTz  (2 matmuls/kh -> [128,1024] psum)
    expT = ScalarE Exp(scale=0.125) -> bf16 sbuf
    flipped ctx: for each 128-q subtile qi: ctq[q,hd+1] += expT-slice
               (stationary, M=128) @ vv[kt][:,h,:] (moving, N=65);
               col 64 = softmax denominator (ones column of vv)
    norm: reciprocal_approx_fast [128,1] + tensor_scalar_mul (per-q denom
               is per-PARTITION in this layout), PE-transpose back to
               ctxn[d, q]
  outT[e,q]  = WoT.T-contract ctxn  (bias bo added host-side)
Projections for e-tiles >= 1 and v-chunks >= 1 are emitted as PE "filler"
groups inside the attention loop so the PE works while ScalarE exps pace
the attention pipeline.
Host: out[b, hq*1024:(hq+1)*1024, :] = outT.T + bo
"""

import numpy as np
import ml_dtypes

import concourse.bacc as bacc
import concourse.tile as tile
from concourse import mybir
from concourse.bass_utils import run_bass_kernel_spmd
from concourse.masks import make_identity

B, S, D = 4, 2048, 1024
H, HD = 16, 64
SQ = 1024          # query rows per core
NDT = D // 128     # 8 d-tiles
NET = D // 128     # 8 e-tiles
NKT = S // 128     # 16 k-tiles
NST = S // 128     # 16 s-tiles
NQC = SQ // 512    # 2 q-chunks per core
BF16 = mybir.dt.bfloat16
F32 = mybir.dt.float32
SCALE = 1.0 / 8.0  # 1/sqrt(HD)

_NC_CACHE = None


def build_nc():
    nc = bacc.Bacc(None, target_bir_lowering=False, debug=True)

    xT_d = nc.declare_dram_parameter("xT", [D, S], BF16, isOutput=False)
    WqT_d = nc.declare_dram_parameter("WqT", [D, D], BF16, isOutput=False)
    WkT_d = nc.declare_dram_parameter("WkT", [D, D], BF16, isOutput=False)
    WvT_d = nc.declare_dram_parameter("WvT", [D, D], BF16, isOutput=False)
    WoT_d = nc.declare_dram_parameter("WoT", [D, D], BF16, isOutput=False)
    bqt_d = nc.declare_dram_parameter("bqt", [128, NET], F32, isOutput=False)
    bkt_d = nc.declare_dram_parameter("bkt", [128, NET], F32, isOutput=False)
    bvr_d = nc.declare_dram_parameter("bvr", [1, D], F32, isOutput=False)
    outT_d = nc.declare_dram_parameter("outT", [D, SQ], F32, isOutput=True)

    VC = 256           # v-projection chunk width (4 heads per chunk)
    NVC = D // VC      # 4 chunks

    with tile.TileContext(nc) as tc:
        with tc.tile_pool(name="resident", bufs=1) as res:
            # ---- resident SBUF tensors ----
            kT = [res.tile([128, S], BF16, name=f"kT{t}", tag=f"kT{t}")
                  for t in range(NET)]
            qTz = [res.tile([128, SQ], BF16, name=f"qTz{h}", tag=f"qTz{h}")
                   for h in range(H)]
            vv = [res.tile([128, H, HD + 1], BF16, name=f"v{t}", tag=f"v{t}")
                  for t in range(NST)]
            ctxn = [[res.tile([128, 512], BF16, name=f"ctxn{qc}_{t}",
                              tag=f"ctxn{qc}_{t}") for t in range(NDT)]
                    for qc in range(NQC)]
            Wo_t = [res.tile([128, D], BF16, name=f"Wo{t}", tag=f"Wo{t}")
                    for t in range(NDT)]
            xT = [res.tile([128, S], BF16, name=f"xT{t}", tag=f"xT{t}")
                  for t in range(NDT)]
            bq_dma = res.tile([128, NET], F32, tag="bq_dma")
            bk_dma = res.tile([128, NET], F32, tag="bk_dma")
            bq_sb = res.tile([128, NET], F32, tag="bq_sb")
            bk_sb = res.tile([128, NET], F32, tag="bk_sb")
            bv_sb = res.tile([1, D], F32, tag="bv_sb")
            bv_bc = res.tile([128, D], F32, tag="bv_bc")
            ident = res.tile([128, 128], BF16, tag="ident")

            nc.sync.dma_start(out=bq_dma, in_=bqt_d[:, :])
            nc.sync.dma_start(out=bk_dma, in_=bkt_d[:, :])
            nc.sync.dma_start(out=bv_sb, in_=bvr_d[:, :])
            # TensorScalarPtr has a single sync-wait slot; route the biases
            # through DVE once so later readers rely on program order.
            nc.vector.tensor_copy(out=bq_sb, in_=bq_dma)
            nc.vector.tensor_copy(out=bk_sb, in_=bk_dma)
            nc.gpsimd.partition_broadcast(bv_bc, bv_sb[0:1, :])
            make_identity(nc, ident)
            for h in range(H):
                z0 = 64 if h % 2 == 0 else 0
                nc.vector.memset(qTz[h][z0:z0 + 64, :], 0.0)
            for t in range(NST):
                # only the denominator column; cols 0:HD are overwritten
                nc.vector.memset(vv[t][:, :, HD:HD + 1], 1.0)

            for t in range(NDT):
                nc.sync.dma_start(out=xT[t], in_=xT_d[t * 128:(t + 1) * 128, :])
            for t in range(NDT):
                nc.sync.dma_start(out=Wo_t[t], in_=WoT_d[t * 128:(t + 1) * 128, :])

            with tc.tile_pool(name="p2", bufs=1) as p2:
                psum_src = {}

                def proj_ps():
                    return psum_src["pool"].tile(
                        [128, 512], F32, name="ps", tag=psum_src["tag"],
                        bufs=psum_src["bufs"])

                # ---------- projection emitters (also used as fillers) ----
                # weight SLICES are DMA-streamed per e-tile/chunk so that
                # three full weight sets never have to live in SBUF at once
                wk_cache = {}
                wq_cache = {}
                wv_cache = {}

                def w_slices(cache, key, W_d, c0, c1, tag, bufs):
                    if key not in cache:
                        ws = []
                        for dt in range(NDT):
                            wt = p2.tile([128, c1 - c0], BF16,
                                         name=f"{tag}{dt}", tag=tag,
                                         bufs=bufs)
                            nc.sync.dma_start(
                                out=wt, in_=W_d[dt * 128:(dt + 1) * 128,
                                                c0:c1])
                            ws.append(wt)
                        cache.clear()
                        cache[key] = ws
                    return cache[key]

                def emit_k_group(et, sc):
                    ws = w_slices(wk_cache, ("k", et), WkT_d,
                                  et * 128, (et + 1) * 128, "wks", 18)
                    ps = proj_ps()
                    for dt in range(NDT):
                        nc.tensor.matmul(
                            ps, ws[dt],
                            xT[dt][:, sc * 512: sc * 512 + 512],
                            start=(dt == 0), stop=(dt == NDT - 1))
                    nc.vector.tensor_scalar_add(
                        out=kT[et][:, sc * 512:(sc + 1) * 512],
                        in0=ps,
                        scalar1=bk_sb[:, et:et + 1])

                def emit_q_group(et, sc):
                    ws = w_slices(wq_cache, ("q", et), WqT_d,
                                  et * 128, (et + 1) * 128, "wqs", 18)
                    ps = proj_ps()
                    for dt in range(NDT):
                        nc.tensor.matmul(
                            ps, ws[dt],
                            xT[dt][:, sc * 512: sc * 512 + 512],
                            start=(dt == 0), stop=(dt == NDT - 1))
                    sl = slice(sc * 512, (sc + 1) * 512)
                    nc.vector.tensor_scalar_add(
                        out=qTz[2 * et][0:64, sl],
                        in0=ps[0:64, :],
                        scalar1=bq_sb[0:64, et:et + 1])
                    nc.vector.tensor_scalar_add(
                        out=qTz[2 * et + 1][64:128, sl],
                        in0=ps[64:128, :],
                        scalar1=bq_sb[64:128, et:et + 1])

                def emit_v_group(st, c):
                    # v chunk c covers e-columns [c*VC, (c+1)*VC) = 4 heads
                    ws = w_slices(wv_cache, ("v", c), WvT_d,
                                  c * VC, (c + 1) * VC, "wvs", 18)
                    psw = proj_ps()
                    ps = psw[:, 0:VC]
                    for dt in range(NDT):
                        nc.tensor.matmul(
                            ps,
                            xT[dt][:, st * 128:(st + 1) * 128],
                            ws[dt],
                            start=(dt == 0), stop=(dt == NDT - 1))
                    nh = VC // HD
                    nc.vector.tensor_add(
                        out=vv[st][:, c * nh:(c + 1) * nh, 0:HD],
                        in0=ps.rearrange("p (h d) -> p h d", h=nh),
                        in1=bv_bc[:, c * VC:(c + 1) * VC].rearrange(
                            "p (h d) -> p h d", h=nh))

                # ---------- upfront: first e-tile + first v chunk ----------
                with tc.psum_pool(name="pf", bufs=1) as pf:
                    psum_src.update(pool=pf, tag="pfg", bufs=2)
                    # warmup: the PE clock needs ~3us of continuous work to
                    # leave the low pstate; spin on the identity tile while
                    # the x DMA is still in flight so the first projection
                    # groups run at full speed
                    warm = pf.tile([128, 128], F32, name="warm", tag="warm",
                                   bufs=1)
                    for _ in range(40):
                        nc.tensor.matmul(warm, ident, ident,
                                         start=True, stop=True)
                    for sc in range(S // 512):
                        emit_k_group(0, sc)
                    emit_q_group(0, 0)
                _cms = [tc.psum_pool(name="sp", bufs=2),
                        tc.psum_pool(name="cp", bufs=2),
                        tc.psum_pool(name="op", bufs=1),
                        tc.psum_pool(name="tp", bufs=1)]
                sp, cp, op, tp = [cm.__enter__() for cm in _cms]
                psum_src.update(pool=op, tag="op", bufs=2)

                fillers = [(emit_v_group, st, 0) for st in range(NST)]
                for et in range(1, NET):
                    for sc in range(S // 512):
                        fillers.append((emit_k_group, et, sc))
                    fillers.append((emit_q_group, et, 0))
                    if et in (2, 4, 6):
                        c = et // 2
                        for st in range(NST):
                            fillers.append((emit_v_group, st, c))
                fillers = fillers[::-1]  # pop from the end
                # q projections for the second q-chunk aren't consumed until
                # qc1 (iteration 128+16*et): drip them into the post-filler
                # bubble where ScalarE paces and the PE has slack
                late_fillers = [(emit_q_group, et, 1) for et in range(NET)]
                late_fillers = late_fillers[::-1]

                # ---------- attention ----------
                def emit_sc(qc, h, kh):
                    ht = h // 2
                    sc_ps = sp.tile([128, 1024], F32, name="sc_ps",
                                    tag="sc", bufs=2)
                    for j in range(2):
                        kt = kh * 2 + j
                        nc.tensor.matmul(
                            sc_ps[:, j * 512:(j + 1) * 512],
                            kT[ht][:, kt * 128:(kt + 1) * 128],
                            qTz[h][:, qc * 512:(qc + 1) * 512],
                            start=True, stop=True)
                    return sc_ps

                def emit_outproj(qc_o, et):
                    ps = op.tile([128, 512], F32, name="ops", tag="op",
                                 bufs=2)
                    for dt in range(NDT):
                        nc.tensor.matmul(
                            ps,
                            Wo_t[dt][:, et * 128:(et + 1) * 128],
                            ctxn[qc_o][dt][:, :],
                            start=(dt == 0), stop=(dt == NDT - 1))
                    osb = p2.tile([128, 512], F32, name="osb", tag="osb",
                                  bufs=2)
                    nc.vector.tensor_copy(out=osb, in_=ps)
                    nc.gpsimd.dma_start(
                        out=outT_d[et * 128:(et + 1) * 128,
                                   qc_o * 512:(qc_o + 1) * 512],
                        in_=osb)

                def emit_norm(ctq_ps, qi, qc, h):
                    # per-q denominators sit per-PARTITION in flipped layout
                    ht, hp = h // 2, (h % 2) * 64
                    inv = p2.tile([128, 1], F32, name="inv", tag="inv",
                                  bufs=3)
                    nc.vector.reciprocal_approx_fast(
                        inv, ctq_ps[:, qi, HD:HD + 1])
                    ctqn = p2.tile([128, HD], BF16, name="ctqn", tag="ctqn",
                                   bufs=3)
                    nc.vector.tensor_scalar_mul(
                        out=ctqn, in0=ctq_ps[:, qi, 0:HD], scalar1=inv)
                    tp_ps = tp.tile([HD, 128], BF16, name="tp_ps", tag="tp",
                                    bufs=1)
                    nc.tensor.transpose(tp_ps, ctqn, ident[:, :])
                    nc.vector.tensor_copy(
                        out=ctxn[qc][ht][hp:hp + HD, qi * 128:(qi + 1) * 128],
                        in_=tp_ps)

                iters = [(qc, h, kh)
                         for qc in range(NQC)
                         for h in range(H)
                         for kh in range(NKT // 2)]
                op_queue = []
                norm_queue = []
                ctq_state = {"ps": None}

                def emit_ctx_step(expT, qc, h, kh):
                    # flipped ctx: expT slice stationary (M=128 q), v moving
                    # (N=65); 4 q-subtiles accumulate in one psum bank
                    if kh == 0:
                        ctq_state["ps"] = cp.tile([128, 4, HD + 1], F32,
                                                  name="ctq", tag="ctq",
                                                  bufs=1)
                    ctq_ps = ctq_state["ps"]
                    for qi in range(4):
                        for j in range(2):
                            kt = kh * 2 + j
                            # start only on the bank's first write: the
                            # start bit marks the WHOLE 2KB bank pending-
                            # zero, so sibling qi regions must not re-set it
                            nc.tensor.matmul(
                                ctq_ps[:, qi, :],
                                expT[:, j * 512 + qi * 128:
                                     j * 512 + qi * 128 + 128],
                                vv[kt][:, h, :],
                                start=(kt == 0 and qi == 0),
                                stop=(kt == NKT - 1))
                    if kh == NKT // 2 - 1:
                        ctq_sb = p2.tile([128, 4, HD + 1], F32,
                                         name="ctq_sb", tag="ctq_sb",
                                         bufs=2)
                        nc.vector.tensor_copy(out=ctq_sb, in_=ctq_ps)
                        for qi in range(4):
                            norm_queue.append((ctq_sb, qi, qc, h))
                        if qc == 0 and h == H - 1:
                            op_queue.extend((0, et) for et in range(NET))

                sc_next = emit_sc(*iters[0])
                delayed = []
                for i, (qc, h, kh) in enumerate(iters):
                    sc_ps = sc_next
                    expT = p2.tile([128, 1024], BF16, name="expT",
                                   tag="expT", bufs=6)
                    nc.scalar.activation(
                        expT, sc_ps,
                        mybir.ActivationFunctionType.Exp)
                    if i + 1 < len(iters):
                        sc_next = emit_sc(*iters[i + 1])
                    delayed.append((expT, qc, h, kh))
                    if len(delayed) > 1:
                        emit_ctx_step(*delayed.pop(0))
                    if norm_queue:
                        emit_norm(*norm_queue.pop(0))
                    for _ in range(2 if i < 16 else 1):
                        if fillers:
                            fn, *args = fillers.pop()
                            fn(*args)
                    if i >= 96 and i % 16 == 0 and late_fillers:
                        fn, *args = late_fillers.pop()
                        fn(*args)
                    if kh == 6 and op_queue and h % 2 == 1:
                        emit_outproj(*op_queue.pop(0))
                while delayed:
                    emit_ctx_step(*delayed.pop(0))
                while norm_queue:
                    emit_norm(*norm_queue.pop(0))
                for args in op_queue:
                    emit_outproj(*args)
                for et in range(NET):
                    emit_outproj(1, et)
                for cm in reversed(_cms):
                    cm.__exit__(None, None, None)
    nc.compile()
    return nc


def _get_nc():
    global _NC_CACHE
    if _NC_CACHE is None:
        _NC_CACHE = build_nc()
    return _NC_CACHE


def _prep_maps(x, Wq, bq, Wk, bk, Wv, bv, Wo):
    bf = ml_dtypes.bfloat16
    WqT = np.ascontiguousarray(Wq.T * SCALE).astype(bf)
    WkT = np.ascontiguousarray(Wk.T).astype(bf)
    WvT = np.ascontiguousarray(Wv.T).astype(bf)
    WoT = np.ascontiguousarray(Wo.T).astype(bf)
    bqt = np.ascontiguousarray(
        bq.reshape(NET, 128).T * SCALE).astype(np.float32)
    bkt = np.ascontiguousarray(bk.reshape(NET, 128).T).astype(np.float32)
    bvr = np.ascontiguousarray(bv.reshape(1, D)).astype(np.float32)
    in_maps = []
    for c in range(8):
        b, hq = c // 2, c % 2
        xTb = np.ascontiguousarray(x[b].T).astype(bf)  # [D, S]
        if hq == 1:
            # rotate so local query half sits at columns [0, SQ)
            xTb = np.ascontiguousarray(
                np.concatenate([xTb[:, SQ:], xTb[:, :SQ]], axis=1))
        in_maps.append(dict(xT=xTb, WqT=WqT, WkT=WkT, WvT=WvT, WoT=WoT,
                            bqt=bqt, bkt=bkt, bvr=bvr))
    return in_maps


def run(x, Wq, bq, Wk, bk, Wv, bv, Wo, bo, trace=False, **spmd_kwargs):
    nc = _get_nc()
    in_maps = _prep_maps(x, Wq, bq, Wk, bk, Wv, bv, Wo)
    res = run_bass_kernel_spmd(nc, in_maps, core_ids=list(range(8)),
                               trace=trace, **spmd_kwargs)
    out = np.empty((B, S, D), np.float32)
    for c in range(8):
        b, hq = c // 2, c % 2
        out[b, hq * SQ:(hq + 1) * SQ, :] = np.asarray(
            res.results[c]["outT"], np.float32).T
    out += bo.astype(np.float32)
    return out, res


def kernel(x, Wq, bq, Wk, bk, Wv, bv, Wo, bo):
    out, _ = run(np.asarray(x, np.float32), np.asarray(Wq, np.float32),
                 np.asarray(bq, np.float32), np.asarray(Wk, np.float32),
                 np.asarray(bk, np.float32), np.asarray(Wv, np.float32),
                 np.asarray(bv, np.float32), np.asarray(Wo, np.float32),
                 np.asarray(bo, np.float32))
    return out
